# revision 15
# baseline (speedup 1.0000x reference)
"""Trainium2 Bass kernel for a 4-layer pre-norm transformer encoder.

Problem: B=4, S=2048, D=256, H=8 heads (DK=32), FF=512, L=4 layers, fp32.

Sharding: token-parallel over B*S across 8 cores. Core c owns batch c//2,
sequence half c%2 (1024 query tokens). Attention needs all 2048 keys of the
batch; each layer exchanges the post-LN1 activations (feature-major bf16)
within same-batch core pairs [[0,1],[2,3],[4,5],[6,7]] via TWO AllGathers,
one per query-half (w) of the tokens, so the second transfer and the K/V
projections it feeds overlap the first attention waves.

The kernel is built around the ScalarE exp wall: softmax exp over the
[keys, queries] score matrix is H*S*T = 16.8M elements/layer/core and
ScalarE (the only exp engine: 1 elem/lane/cycle @1.2GHz) is the pipeline
floor at ~147us/layer. Everything else is arranged to hide under it:
 - passes run w-major ((c0,w0),(c1,w0),(c0,w1),(c1,w1)) so the w0 tokens
   finish attention halfway through the layer; their normalize + O-proj +
   residual + LN2 + FFN ("postproc") is emitted AFTER the w1 passes so the
   dataflow Tile scheduler runs it in the PE/DVE gaps of the w1 exp stream.
 - the next layer's LN1/transpose/bounce/AllGather/Q/K/V for its w0 tokens
   depends only on postproc(w0), so it also fills the w1 shadows, and the
   next layer's first waves start right after this layer's last exp.
 - key blocks are processed in w0-blocks-first order (global token groups
   [0:512] and [1024:1536]) so waves 0-7 of every pass depend only on the
   first AllGather.
 - ALL matmul evacuations, biases, relu and softmax-denominator reciprocal
   (reciprocal_approx_fast) run on DVE; ScalarE keeps only exp and the tiny
   rstd Ln/Exp chains (emitted with high priority so they preempt the
   pending-exp queue instead of draining behind it).
 - matmul operands bf16 (full fp32 PSUM accumulation), LN scale/bias and
   the 1/sqrt(DK) folded into weights host-side; rstd via exp(-0.5*ln(v+e))
   keeps ScalarE on the natural_log_exp table set.
 - scores computed transposed S^T [keys, queries]; per wave the 4 heads'
   score matmuls stream concurrently through distinct PE row-groups
   (tile_position); A@V uses lhsT = [V | ones] so row 32 of each head
   accumulator carries the softmax denominator for free, head pairs run as
   column-tiled concurrent matmuls.
 - layer 0 skips LN1 entirely when ln0 is identity (LN of an
   already-normalized vector: rstd differs from 1 by O(eps)).
"""
import sys

sys.path.insert(0, "/opt/trn_rl_repo")

import numpy as np

import concourse.bass as bass
import concourse.mybir as mybir
import concourse.tile as tile
from concourse.bass_utils import run_bass_kernel_spmd
from concourse.masks import make_identity


# ---- problem constants (hardcoded per contract) ----
B, S, D, H, L, FF = 4, 2048, 256, 8, 4, 512
DK = D // H          # 32
EPS = 1e-5
NC = 8               # cores
T = (B * S) // NC    # 1024 own tokens per core
NT = T // 128        # 8 token tiles
SK = S               # 2048 keys
NKB = SK // 128      # 16 key blocks
F32 = mybir.dt.float32
BF16 = mybir.dt.bfloat16

# key blocks in w0-tokens-first global order: [0:512] and [1024:1536] first
KB_ORDER = [0, 1, 2, 3, 8, 9, 10, 11, 4, 5, 6, 7, 12, 13, 14, 15]

# weight-concat layout offsets (floats per partition, per layer)
QOFF, KOFF, VOFF, OOFF, W1OFF, W2OFF = 0, 512, 1024, 1552, 2064, 3088
WFREE = 4112
# bias-concat layout: bq(2) bk(2) bo(2) b1(4) b2(2) bv_bc(264)
BQOFF, BKOFF, BOOFF, B1OFF, B2OFF, BVOFF = 0, 2, 4, 6, 10, 12
BFREE = 276


def dram_bcast(ap, p=128):
    """broadcast a DRAM AP across p partitions (stride-0 leading dim)"""
    return bass.AP(tensor=ap.tensor, offset=ap.offset, ap=[[0, p]] + list(ap.ap))


def build_nc(ln0_identity=False, split_tail=True):
    nc = bass.Bass("TRN2", num_devices=NC)

    x_in = nc.declare_dram_parameter("x_sh", [T, D], F32, isOutput=False)
    wcat = nc.declare_dram_parameter("wcat", [L, 128, WFREE], BF16, isOutput=False)
    bcat = nc.declare_dram_parameter("bcat", [L, 128, BFREE], F32, isOutput=False)
    bvcat = nc.declare_dram_parameter("bvcat", [L, 264], BF16, isOutput=False)
    ln0s_in = nc.declare_dram_parameter("ln0_s", [D], F32, isOutput=False)
    ln0b_in = nc.declare_dram_parameter("ln0_b", [D], F32, isOutput=False)
    y_out = nc.declare_dram_parameter("y", [T, D], F32, isOutput=True)

    with tile.TileContext(nc) as tc:
        build_body(nc, tc, x_in, wcat, bcat, bvcat, ln0s_in, ln0b_in, y_out,
                   ln0_identity)

    if split_tail:
        _split_tail_waits(nc)
    return nc


def _split_tail_waits(nc):
    """walrus's TPB_CTRL lowering supports only one sync-wait command per
    instruction, but the TileContext kernel-tail drain aggregates one wait
    per outstanding proc lane. A chain of same-engine single-wait NoOps
    gates identically, so rewrite the tail block that way."""
    cnt = [0]

    def mk_carrier(engine, wait):
        ins = mybir.InstNoOp(name=f"waitfix-{cnt[0]}", ins=[], outs=[])
        cnt[0] += 1
        ins.engine = engine
        ins.sync_info = mybir.SyncInfo(on_wait=[wait], on_update=[])
        return ins

    def needs_split(ins):
        si = ins.sync_info
        return si is not None and len(si.on_wait) > 1

    for bb in nc.main_func.blocks:
        insts = list(bb.instructions)
        if not any(needs_split(ins) for ins in insts):
            continue
        out = []
        for ins in insts:
            si = ins.sync_info
            if needs_split(ins):
                waits = list(si.on_wait)
                for w in waits[:-1]:
                    out.append(mk_carrier(ins.engine, w))
                ins.sync_info = mybir.SyncInfo(
                    on_wait=waits[-1:], on_update=list(si.on_update)
                )
            out.append(ins)
        bb.instructions = out


def build_body(nc, tc, x_in, wcat, bcat, bvcat, ln0s_in, ln0b_in, y_out,
               ln0_identity=False):
    import contextlib

    ctx = contextlib.ExitStack()
    with ctx:
        # ---- pools ----
        singles = ctx.enter_context(tc.tile_pool(name="singles", bufs=1))
        wpool = ctx.enter_context(tc.tile_pool(name="wpool", bufs=2))
        bpool = ctx.enter_context(tc.tile_pool(name="bpool", bufs=2))
        big = ctx.enter_context(tc.tile_pool(name="big", bufs=3))
        fm = ctx.enter_context(tc.tile_pool(name="fm", bufs=2))     # x2wT
        qp = ctx.enter_context(tc.tile_pool(name="qp", bufs=2))     # qT
        kv = ctx.enter_context(tc.tile_pool(name="kv", bufs=2))     # kT/x2full
        vpool = ctx.enter_context(tc.tile_pool(name="vpool", bufs=2))
        oraw = ctx.enter_context(tc.tile_pool(name="oraw", bufs=2))
        rb = ctx.enter_context(tc.tile_pool(name="rb", bufs=1))
        exps_pool = ctx.enter_context(tc.tile_pool(name="exps", bufs=3))
        stat = ctx.enter_context(tc.tile_pool(name="stat", bufs=4))
        dpool = ctx.enter_context(tc.tile_pool(name="dpool", bufs=1))
        # PSUM budget (8 banks): 3 rotating [128,1024] fp32 score slots
        # (6 banks) + 2 A@V accumulator banks. Projections / transposes /
        # FFN rotate through the score slots.
        ps = ctx.enter_context(tc.tile_pool(name="ps", bufs=1, space="PSUM"))
        accp = ctx.enter_context(tc.tile_pool(name="accp", bufs=1, space="PSUM"))
        dram = ctx.enter_context(tc.tile_pool(name="dram", bufs=2, space="DRAM"))

        _psc = [0]

        def pstile(shape, dtype):
            _psc[0] += 1
            return ps.tile(shape, dtype, name="pst",
                           tag=f"s2{_psc[0] % 3}",
                           padded_shape=[128, 1024])

        # ---- persistent singles ----
        identity = singles.tile([128, 128], BF16)
        make_identity(nc, identity)
        epsc = singles.tile([128, 1], F32)
        nc.vector.memset(epsc, EPS)
        h_t = singles.tile([128, NT, D], F32)
        ln0s_t = singles.tile([128, D], F32)
        ln0b_t = singles.tile([128, D], F32)
        nc.sync.dma_start(out=ln0s_t, in_=dram_bcast(ln0s_in.ap()))
        nc.sync.dma_start(out=ln0b_t, in_=dram_bcast(ln0b_in.ap()))

        def ln_stats_apply(src, dst, tiles, dst_off=0):
            """LayerNorm (stats + normalize, no scale/bias) of `src` token
            tiles [128, len(tiles), D] into dst[:, dst_off+i, :]. Stats on
            DVE; rstd Ln/Exp on ScalarE at high priority (tiny, must not
            queue behind pending exps)."""
            n = len(tiles)
            mvs = stat.tile([128, n, 2], F32, tag="mvs")
            rstd = stat.tile([128, n], F32, tag="rstd")
            for i in range(n):
                st = stat.tile([128, 6], F32, tag="bnstats")
                nc.vector.bn_stats(out=st, in_=src[:, tiles[i], :])
                nc.vector.bn_aggr(out=mvs[:, i, :], in_=st)
            with tc.high_priority():
                nc.scalar.activation(out=rstd, in_=mvs[:, :, 1],
                                     func=mybir.ActivationFunctionType.Ln,
                                     bias=epsc[:, 0:1])
                nc.scalar.activation(out=rstd, in_=rstd,
                                     func=mybir.ActivationFunctionType.Exp,
                                     scale=-0.5)
            for i, t in enumerate(tiles):
                nc.vector.tensor_scalar(
                    out=dst[:, dst_off + i, :], in0=src[:, t, :],
                    scalar1=mvs[:, i, 0:1], scalar2=rstd[:, i:i + 1],
                    op0=mybir.AluOpType.subtract, op1=mybir.AluOpType.mult)

        # ---- LN0: h = ln0(x), per w half ----
        x0 = big.tile([128, NT, D], F32, tag="x0")
        x0r = x_in.ap().rearrange("(t p) d -> p t d", p=128)
        for hf in range(2):
            nc.sync.dma_start(out=x0[:, 4 * hf:4 * hf + 4, :],
                              in_=x0r[:, 4 * hf:4 * hf + 4, :])
        for w in range(2):
            ln_stats_apply(x0, h_t, [4 * w + i for i in range(4)],
                           dst_off=4 * w)
            if not ln0_identity:
                for t in range(4 * w, 4 * w + 4):
                    nc.vector.tensor_mul(out=h_t[:, t, :], in0=h_t[:, t, :],
                                         in1=ln0s_t)
                    nc.vector.tensor_add(out=h_t[:, t, :], in0=h_t[:, t, :],
                                         in1=ln0b_t)

        # ---- layers ----
        for l in range(L):
            wt = wpool.tile([128, WFREE], BF16)
            nc.sync.dma_start(out=wt, in_=wcat[l, :, :])
            bt = bpool.tile([128, BFREE], F32)
            nc.sync.dma_start(out=bt, in_=bcat[l, :, :])
            bvbc_t = bpool.tile([128, 264], BF16, tag="bvbc")
            nc.sync.dma_start(out=bvbc_t, in_=dram_bcast(bvcat[l, :]))

            def wq_sl(ci, co):
                return wt[:, QOFF + ci * 256 + co * 128: QOFF + ci * 256 + co * 128 + 128]

            def wk_sl(ci, co):
                return wt[:, KOFF + ci * 256 + co * 128: KOFF + ci * 256 + co * 128 + 128]

            def wv_sl(ci):
                return wt[:, VOFF + ci * 264: VOFF + ci * 264 + 264]

            def wo_sl(ci, co):
                return wt[:, OOFF + ci * 256 + co * 128: OOFF + ci * 256 + co * 128 + 128]

            def w1_sl(ci, co):
                return wt[:, W1OFF + ci * 512 + co * 128: W1OFF + ci * 512 + co * 128 + 128]

            def w2_sl(ci, co):
                return wt[:, W2OFF + ci * 256 + co * 128: W2OFF + ci * 256 + co * 128 + 128]

            # per-layer persistent tiles
            x2wT = [None, None]
            qT = qp.tile([128, 2, T], BF16, tag="qt")
            kT = kv.tile([128, 2, 4, 512], BF16, tag="kt")
            x2full = kv.tile([128, 2, 4, 512], BF16, tag="x2full")
            v_t = vpool.tile([128, NKB, 264], BF16, tag="v")
            o_t = oraw.tile([128, 2, T], BF16, tag="oraw")
            denoms = dpool.tile([128, 2, 1024], F32, tag="denoms")
            rbt = rb.tile([128, 2, T], F32, tag="rb")
            rdram = dram.tile([4, 2, 1024], F32, tag="rdram")

            skip_ln1 = (l == 0) and ln0_identity

            def prep_w(w):
                """LN1(w) -> x2 -> transpose -> bounce -> AllGather(w) ->
                x2full(w-groups) -> Q(w), K(w-groups), V(w-blocks)."""
                x2w = big.tile([128, 4, D], BF16, tag="x2w")
                if skip_ln1:
                    # LN1 of an LN0-normalized vector is identity to O(eps);
                    # just downcast h for the feature-major matmuls.
                    for i in range(4):
                        nc.vector.tensor_copy(out=x2w[:, i, :],
                                              in_=h_t[:, 4 * w + i, :])
                else:
                    ln_stats_apply(h_t, x2w, [4 * w + i for i in range(4)])
                x2src, toff = x2w, 0
                # transpose own w-half to feature-major
                x2wT[w] = fm.tile([128, 2, 512], BF16, name="x2wT", tag="x2wT")
                bounce_in = dram.tile([D, 512], BF16, tag="bin")
                bounce_out = dram.tile([2 * D, 512], BF16, tag="bout")
                for c in range(2):
                    pT = pstile([128, 512], BF16)
                    for t4 in range(4):
                        nc.tensor.transpose(
                            pT[:, 128 * t4:128 * (t4 + 1)],
                            x2src[:, toff + t4, 128 * c:128 * (c + 1)],
                            identity)
                    nc.vector.tensor_copy(out=x2wT[w][:, c, :], in_=pT)
                    nc.sync.dma_start(out=bounce_in[128 * c:128 * (c + 1), :],
                                      in_=x2wT[w][:, c, :])
                nc.gpsimd.collective_compute(
                    "AllGather", mybir.AluOpType.bypass,
                    replica_groups=[[0, 1], [2, 3], [4, 5], [6, 7]],
                    ins=[bounce_in.opt()], outs=[bounce_out.opt()])
                for g in range(2):
                    for c in range(2):
                        nc.sync.dma_start(
                            out=x2full[:, c, 2 * g + w, :],
                            in_=bounce_out[D * g + 128 * c:
                                           D * g + 128 * (c + 1), :])
                # Q projection (own tokens, this w half)
                for co in range(2):
                    pq = pstile([128, 512], F32)
                    for ci in range(2):
                        nc.tensor.matmul(pq, wq_sl(ci, co), x2wT[w][:, ci, :],
                                         start=(ci == 0), stop=(ci == 1))
                    nc.vector.tensor_scalar_add(
                        out=qT[:, co, 512 * w:512 * (w + 1)], in0=pq,
                        scalar1=bt[:, BQOFF + co:BQOFF + co + 1])
                # K projection for this w's token groups (global order:
                # groups w and w+2 of kT's dim-2)
                for co in range(2):
                    pk = pstile([128, 1024], F32)
                    for g in range(2):
                        for ci in range(2):
                            nc.tensor.matmul(
                                pk[:, 512 * g:512 * (g + 1)], wk_sl(ci, co),
                                x2full[:, ci, 2 * g + w, :],
                                start=(ci == 0), stop=(ci == 1))
                    for g in range(2):
                        nc.vector.tensor_scalar_add(
                            out=kT[:, co, 2 * g + w, :],
                            in0=pk[:, 512 * g:512 * (g + 1)],
                            scalar1=bt[:, BKOFF + co:BKOFF + co + 1])
                # V projection (token-major) for this w's key blocks
                for kb in KB_ORDER[8 * w:8 * w + 8]:
                    pv = pstile([128, 264], F32)
                    for ci in range(2):
                        nc.tensor.matmul(
                            pv, x2full[:, ci, kb // 4, 128 * (kb % 4):
                                       128 * (kb % 4) + 128],
                            wv_sl(ci), start=(ci == 0), stop=(ci == 1))
                    nc.vector.tensor_add(out=v_t[:, kb, :], in0=pv,
                                         in1=bvbc_t)

            def attn_pass(chunk, w, mid_cb=None):
                q0 = 512 * w
                pacc0 = accp.tile([128, 512], F32, tag="acc0")
                pacc1 = accp.tile([128, 512], F32, tag="acc1")
                for ki, kb in enumerate(KB_ORDER):
                    if ki == 8 and mid_cb is not None:
                        # K/V for the second token-half must be emitted
                        # before the waves that consume them (program order
                        # carries the data deps); placing them here also
                        # ring-orders their PSUM slots between the wave
                        # slots so nothing deadlocks.
                        mid_cb()
                    sts = []
                    ets = []
                    for j in range(2):
                        sts.append(ps.tile(
                            [128, 1024], F32, name="sps",
                            tag=f"s2{(2 * ki + j) % 3}"))
                        ets.append(exps_pool.tile(
                            [128, 1024], BF16, name="et", tag="exps", bufs=3))
                    for hh in range(4):
                        nc.tensor.matmul(
                            sts[hh // 2][:, 512 * (hh % 2):
                                         512 * (hh % 2) + 512],
                            kT[32 * hh:32 * hh + 32, chunk, kb // 4,
                               128 * (kb % 4):128 * (kb % 4) + 128],
                            qT[32 * hh:32 * hh + 32, chunk, q0:q0 + 512],
                            start=True, stop=True,
                            tile_position=(32 * hh, 0))
                    for j in range(2):
                        nc.scalar.activation(
                            out=ets[j], in_=sts[j],
                            func=mybir.ActivationFunctionType.Exp)
                    for hh in range(4):
                        pacc = pacc0 if hh < 2 else pacc1
                        base = 64 * (hh % 2)
                        hd = 4 * chunk + hh
                        nc.tensor.matmul(
                            pacc[base:base + 33, :],
                            v_t[:, kb, 33 * hd:33 * hd + 33],
                            ets[hh // 2][:, 512 * (hh % 2):
                                         512 * (hh % 2) + 512],
                            start=(ki == 0), stop=(ki == NKB - 1),
                            tile_position=(0, base), skip_group_check=True)
                # evacuate heads: O rows + denominator rows (all DVE)
                for hh in range(4):
                    pacc = pacc0 if hh < 2 else pacc1
                    base = 64 * (hh % 2)
                    nc.vector.tensor_copy(
                        out=o_t[32 * hh:32 * hh + 32, chunk, q0:q0 + 512],
                        in_=pacc[base:base + 32, :])
                    nc.vector.tensor_copy(
                        out=denoms[32 * hh:32 * hh + 1, chunk, q0:q0 + 512],
                        in_=pacc[base + 32:base + 33, :])
                nc.sync.dma_start(out=rdram[:, chunk, q0:q0 + 512],
                                  in_=denoms[::32, chunk, q0:q0 + 512])
                for hh in range(4):
                    nc.sync.dma_start(
                        out=rbt[32 * hh:32 * hh + 32, chunk, q0:q0 + 512],
                        in_=dram_bcast(rdram[hh, chunk, q0:q0 + 512], 32))
                rsl = rbt[:, chunk, q0:q0 + 512]
                nc.vector.reciprocal(out=rsl, in_=rsl)

            def postproc_w(w):
                """normalize happened already; O-proj + residual + LN2 +
                FFN + residual for the 4 token tiles of half w."""
                q0 = 512 * w
                # ---- output projection ----
                attnU = big.tile([128, 2, 512], BF16, tag="attnU")
                for co in range(2):
                    po = pstile([128, 512], F32)
                    for ci in range(2):
                        nc.tensor.matmul(po, wo_sl(ci, co),
                                         o_t[:, ci, q0:q0 + 512],
                                         start=(ci == 0), stop=(ci == 1))
                    nc.vector.tensor_scalar_add(
                        out=attnU[:, co, :], in0=po,
                        scalar1=bt[:, BOOFF + co:BOOFF + co + 1])
                # transpose to token-major + residual add
                pT = pstile([128, 1024], BF16)
                for t4 in range(4):
                    for c in range(2):
                        nc.tensor.transpose(
                            pT[:, 256 * t4 + 128 * c:256 * t4 + 128 * (c + 1)],
                            attnU[:, c, 128 * t4:128 * (t4 + 1)], identity)
                for t4 in range(4):
                    t = 4 * w + t4
                    nc.vector.tensor_add(out=h_t[:, t, :], in0=h_t[:, t, :],
                                         in1=pT[:, 256 * t4:256 * (t4 + 1)])
                # ---- FFN (ln2 folded into w1/b1) ----
                x2f = big.tile([128, 4, D], BF16, tag="x2f")
                ln_stats_apply(h_t, x2f, [4 * w + i for i in range(4)])
                x2fT = fm.tile([128, 2, 512], BF16, tag="x2fT")
                for c in range(2):
                    pTT = pstile([128, 512], BF16)
                    for t4 in range(4):
                        nc.tensor.transpose(
                            pTT[:, 128 * t4:128 * (t4 + 1)],
                            x2f[:, t4, 128 * c:128 * (c + 1)], identity)
                    nc.vector.tensor_copy(out=x2fT[:, c, :], in_=pTT)
                h1 = big.tile([128, 4, 512], BF16, tag="h1")
                for co in range(4):
                    p1 = pstile([128, 512], F32)
                    for ci in range(2):
                        nc.tensor.matmul(p1, w1_sl(ci, co), x2fT[:, ci, :],
                                         start=(ci == 0), stop=(ci == 1))
                    # bias + relu fused on DVE
                    nc.vector.tensor_scalar(
                        out=h1[:, co, :], in0=p1,
                        scalar1=bt[:, B1OFF + co:B1OFF + co + 1], scalar2=0.0,
                        op0=mybir.AluOpType.add, op1=mybir.AluOpType.max)
                ffnU = big.tile([128, 2, 512], BF16, tag="ffnU")
                for co in range(2):
                    p2 = pstile([128, 512], F32)
                    for ci in range(4):
                        nc.tensor.matmul(p2, w2_sl(ci, co), h1[:, ci, :],
                                         start=(ci == 0), stop=(ci == 3))
                    nc.vector.tensor_scalar_add(
                        out=ffnU[:, co, :], in0=p2,
                        scalar1=bt[:, B2OFF + co:B2OFF + co + 1])
                pT2 = pstile([128, 1024], BF16)
                for t4 in range(4):
                    for c in range(2):
                        nc.tensor.transpose(
                            pT2[:, 256 * t4 + 128 * c:256 * t4 + 128 * (c + 1)],
                            ffnU[:, c, 128 * t4:128 * (t4 + 1)], identity)
                for t4 in range(4):
                    t = 4 * w + t4
                    nc.vector.tensor_add(out=h_t[:, t, :], in0=h_t[:, t, :],
                                         in1=pT2[:, 256 * t4:256 * (t4 + 1)])
                if l == L - 1:
                    yr = y_out.ap().rearrange("(t p) d -> p t d", p=128)
                    nc.sync.dma_start(out=yr[:, 4 * w:4 * w + 4, :],
                                      in_=h_t[:, 4 * w:4 * w + 4, :])

            # ---- layer emission order (priority = emission order; the
            # dataflow scheduler overlaps across it by deps) ----
            prep_w(0)
            attn_pass(0, 0, mid_cb=lambda: prep_w(1))
            attn_pass(1, 0)
            # w0 normalize after both w0 chunks
            nc.vector.tensor_mul(out=o_t[:, :, 0:512], in0=o_t[:, :, 0:512],
                                 in1=rbt[:, :, 0:512])
            attn_pass(0, 1)
            attn_pass(1, 1)
            nc.vector.tensor_mul(out=o_t[:, :, 512:1024],
                                 in0=o_t[:, :, 512:1024],
                                 in1=rbt[:, :, 512:1024])
            # postproc(w0) runs in the shadow of the w1 passes; postproc(w1)
            # is the layer tail and overlaps the next layer's prep.
            postproc_w(0)
            postproc_w(1)


# ---------------------------------------------------------------------------
# host side
# ---------------------------------------------------------------------------
_NC_CACHE = {}


def _get_nc(ln0_identity=False):
    if ln0_identity not in _NC_CACHE:
        _NC_CACHE[ln0_identity] = build_nc(ln0_identity)
    return _NC_CACHE[ln0_identity]


def _ln0_identity(inputs):
    return bool(
        np.all(np.asarray(inputs["ln0_s"], np.float32) == 1.0)
        and np.all(np.asarray(inputs["ln0_b"], np.float32) == 0.0))


def _prep_host(inputs):
    """Fold LN scales/biases + softmax scale into weights; build concat layouts."""
    f = lambda k: np.asarray(inputs[k], np.float32)
    wq, wk, wv, wo = f("wq"), f("wk"), f("wv"), f("wo")
    w1, w2 = f("w1"), f("w2")
    bq, bk, bv, bo = f("bq"), f("bk"), f("bv"), f("bo")
    b1, b2 = f("b1"), f("b2")
    l1s, l1b = f("ln1_s"), f("ln1_b")
    l2s, l2b = f("ln2_s"), f("ln2_b")

    sc = 1.0 / np.sqrt(np.float32(DK))
    wcat = np.zeros((L, 128, WFREE), np.float32)
    bcat = np.zeros((L, 128, BFREE), np.float32)
    bvcat = np.zeros((L, 264), np.float32)
    for l in range(L):
        wq_f = (l1s[l][:, None] * wq[l]) * sc
        bq_f = (l1b[l] @ wq[l] + bq[l]) * sc
        wk_f = l1s[l][:, None] * wk[l]
        bk_f = l1b[l] @ wk[l] + bk[l]
        wv_f = l1s[l][:, None] * wv[l]
        bv_f = l1b[l] @ wv[l] + bv[l]
        w1_f = l2s[l][:, None] * w1[l]
        b1_f = l2b[l] @ w1[l] + b1[l]

        # interleave wv columns into 33-wide head groups with a ones-slot
        wv_aug = np.zeros((D, 264), np.float32)
        bv_aug = np.zeros((264,), np.float32)
        for hd in range(H):
            wv_aug[:, 33 * hd:33 * hd + 32] = wv_f[:, 32 * hd:32 * hd + 32]
            bv_aug[33 * hd:33 * hd + 32] = bv_f[32 * hd:32 * hd + 32]
            bv_aug[33 * hd + 32] = 1.0  # ones column -> denominator row

        def chunks(w, width):
            n_ci = w.shape[0] // 128
            return np.concatenate(
                [w[128 * ci:128 * (ci + 1), :] for ci in range(n_ci)], axis=1)

        wcat[l, :, QOFF:QOFF + 512] = chunks(wq_f, 256)
        wcat[l, :, KOFF:KOFF + 512] = chunks(wk_f, 256)
        wcat[l, :, VOFF:VOFF + 528] = chunks(wv_aug, 264)
        wcat[l, :, OOFF:OOFF + 512] = chunks(wo[l], 256)
        wcat[l, :, W1OFF:W1OFF + 1024] = chunks(w1_f, 512)
        wcat[l, :, W2OFF:W2OFF + 1024] = chunks(w2[l], 256)

        for co in range(2):
            bcat[l, :, BQOFF + co] = bq_f[128 * co:128 * (co + 1)]
            bcat[l, :, BKOFF + co] = bk_f[128 * co:128 * (co + 1)]
            bcat[l, :, BOOFF + co] = bo[l][128 * co:128 * (co + 1)]
            bcat[l, :, B2OFF + co] = b2[l][128 * co:128 * (co + 1)]
        for co in range(4):
            bcat[l, :, B1OFF + co] = b1_f[128 * co:128 * (co + 1)]
        bvcat[l] = bv_aug

    import ml_dtypes

    return wcat.astype(ml_dtypes.bfloat16), bcat, bvcat.astype(ml_dtypes.bfloat16)


def kernel(**inputs):
    nc = _get_nc(_ln0_identity(inputs))
    wcat, bcat, bvcat = _prep_host(inputs)
    x = np.asarray(inputs["x"], np.float32)
    ln0_s = np.asarray(inputs["ln0_s"], np.float32)
    ln0_b = np.asarray(inputs["ln0_b"], np.float32)

    in_maps = []
    for c in range(NC):
        b, half = c // 2, c % 2
        in_maps.append({
            "x_sh": np.ascontiguousarray(x[b, half * T:(half + 1) * T, :]),
            "wcat": wcat, "bcat": bcat, "bvcat": bvcat,
            "ln0_s": ln0_s, "ln0_b": ln0_b,
        })

    res = run_bass_kernel_spmd(nc, in_maps, core_ids=list(range(NC)))
    out = np.zeros((B, S, D), np.float32)
    for c in range(NC):
        b, half = c // 2, c % 2
        out[b, half * T:(half + 1) * T, :] = res.results[c]["y"]
    return out


# revision 19
# speedup vs baseline: 1.1571x; 1.1571x over previous
"""Trainium2 Bass kernel for a 4-layer pre-norm transformer encoder.

Problem: B=4, S=2048, D=256, H=8 heads (DK=32), FF=512, L=4 layers, fp32.

Sharding: token-parallel over B*S across 8 cores. Core c owns batch c//2,
sequence half c%2 (1024 query tokens). Attention needs all 2048 keys of the
batch; each layer exchanges the post-LN1 activations (feature-major bf16)
within same-batch core pairs [[0,1],[2,3],[4,5],[6,7]] via TWO AllGathers,
one per query-half (w) of the tokens, so the second transfer and the K/V
projections it feeds overlap the first attention waves.

The kernel is built around the ScalarE exp wall: softmax exp over the
[keys, queries] score matrix is H*S*T = 16.8M elements/layer/core and
ScalarE (the only exp engine: 1 elem/lane/cycle @1.2GHz) is the pipeline
floor at ~147us/layer. Everything else is arranged to hide under it:
 - passes run w-major ((c0,w0),(c1,w0),(c0,w1),(c1,w1)) so the w0 tokens
   finish attention halfway through the layer; their normalize + O-proj +
   residual + LN2 + FFN ("postproc") is emitted AFTER the w1 passes so the
   dataflow Tile scheduler runs it in the PE/DVE gaps of the w1 exp stream.
 - the next layer's LN1/transpose/bounce/AllGather/Q/K/V for its w0 tokens
   depends only on postproc(w0), so it also fills the w1 shadows, and the
   next layer's first waves start right after this layer's last exp.
 - key blocks are processed in w0-blocks-first order (global token groups
   [0:512] and [1024:1536]) so waves 0-7 of every pass depend only on the
   first AllGather.
 - ALL matmul evacuations, biases, relu and softmax-denominator reciprocal
   (reciprocal_approx_fast) run on DVE; ScalarE keeps only exp and the tiny
   rstd Ln/Exp chains (emitted with high priority so they preempt the
   pending-exp queue instead of draining behind it).
 - matmul operands bf16 (full fp32 PSUM accumulation), LN scale/bias and
   the 1/sqrt(DK) folded into weights host-side; rstd via exp(-0.5*ln(v+e))
   keeps ScalarE on the natural_log_exp table set.
 - scores computed transposed S^T [keys, queries]; per wave the 4 heads'
   score matmuls stream concurrently through distinct PE row-groups
   (tile_position); A@V uses lhsT = [V | ones] so row 32 of each head
   accumulator carries the softmax denominator for free, head pairs run as
   column-tiled concurrent matmuls.
 - layer 0 skips LN1 entirely when ln0 is identity (LN of an
   already-normalized vector: rstd differs from 1 by O(eps)).
"""
import sys

sys.path.insert(0, "/opt/trn_rl_repo")

import numpy as np

import concourse.bass as bass
import concourse.mybir as mybir
import concourse.tile as tile
from concourse.bass_utils import run_bass_kernel_spmd
from concourse.masks import make_identity


# ---- problem constants (hardcoded per contract) ----
B, S, D, H, L, FF = 4, 2048, 256, 8, 4, 512
DK = D // H          # 32
EPS = 1e-5
NC = 8               # cores
T = (B * S) // NC    # 1024 own tokens per core
NT = T // 128        # 8 token tiles
SK = S               # 2048 keys
NKB = SK // 128      # 16 key blocks
F32 = mybir.dt.float32
BF16 = mybir.dt.bfloat16

# key blocks in w0-tokens-first global order: [0:512] and [1024:1536] first
KB_ORDER = [0, 1, 2, 3, 8, 9, 10, 11, 4, 5, 6, 7, 12, 13, 14, 15]

# weight-concat layout offsets (floats per partition, per layer)
QOFF, KOFF, VOFF, OOFF, W1OFF, W2OFF = 0, 512, 1024, 1552, 2064, 3088
WFREE = 4112
# bias-concat layout: bq(2) bk(2) bo(2) b1(4) b2(2) bv_bc(264)
BQOFF, BKOFF, BOOFF, B1OFF, B2OFF, BVOFF = 0, 2, 4, 6, 10, 12
BFREE = 276


def dram_bcast(ap, p=128):
    """broadcast a DRAM AP across p partitions (stride-0 leading dim)"""
    return bass.AP(tensor=ap.tensor, offset=ap.offset, ap=[[0, p]] + list(ap.ap))


def build_nc(ln0_identity=False, split_tail=True):
    nc = bass.Bass("TRN2", num_devices=NC)

    x_in = nc.declare_dram_parameter("x_sh", [T, D], F32, isOutput=False)
    wcat = nc.declare_dram_parameter("wcat", [L, 128, WFREE], BF16, isOutput=False)
    bcat = nc.declare_dram_parameter("bcat", [L, 128, BFREE], F32, isOutput=False)
    bvcat = nc.declare_dram_parameter("bvcat", [L, 264], BF16, isOutput=False)
    ln0s_in = nc.declare_dram_parameter("ln0_s", [D], F32, isOutput=False)
    ln0b_in = nc.declare_dram_parameter("ln0_b", [D], F32, isOutput=False)
    y_out = nc.declare_dram_parameter("y", [T, D], F32, isOutput=True)

    with tile.TileContext(nc) as tc:
        build_body(nc, tc, x_in, wcat, bcat, bvcat, ln0s_in, ln0b_in, y_out,
                   ln0_identity)

    if split_tail:
        _split_tail_waits(nc)
    return nc


def _split_tail_waits(nc):
    """walrus's TPB_CTRL lowering supports only one sync-wait command per
    instruction, but the TileContext kernel-tail drain aggregates one wait
    per outstanding proc lane. A chain of same-engine single-wait NoOps
    gates identically, so rewrite the tail block that way."""
    cnt = [0]

    def mk_carrier(engine, wait):
        ins = mybir.InstNoOp(name=f"waitfix-{cnt[0]}", ins=[], outs=[])
        cnt[0] += 1
        ins.engine = engine
        ins.sync_info = mybir.SyncInfo(on_wait=[wait], on_update=[])
        return ins

    def needs_split(ins):
        si = ins.sync_info
        return si is not None and len(si.on_wait) > 1

    for bb in nc.main_func.blocks:
        insts = list(bb.instructions)
        if not any(needs_split(ins) for ins in insts):
            continue
        out = []
        for ins in insts:
            si = ins.sync_info
            if needs_split(ins):
                waits = list(si.on_wait)
                for w in waits[:-1]:
                    out.append(mk_carrier(ins.engine, w))
                ins.sync_info = mybir.SyncInfo(
                    on_wait=waits[-1:], on_update=list(si.on_update)
                )
            out.append(ins)
        bb.instructions = out


def build_body(nc, tc, x_in, wcat, bcat, bvcat, ln0s_in, ln0b_in, y_out,
               ln0_identity=False):
    import contextlib

    ctx = contextlib.ExitStack()
    with ctx:
        # ---- pools ----
        singles = ctx.enter_context(tc.tile_pool(name="singles", bufs=1))
        wpool = ctx.enter_context(tc.tile_pool(name="wpool", bufs=2))
        bpool = ctx.enter_context(tc.tile_pool(name="bpool", bufs=2))
        big = ctx.enter_context(tc.tile_pool(name="big", bufs=3))
        fm = ctx.enter_context(tc.tile_pool(name="fm", bufs=2))     # x2wT
        qp = ctx.enter_context(tc.tile_pool(name="qp", bufs=2))     # qT
        kv = ctx.enter_context(tc.tile_pool(name="kv", bufs=2))     # kT/x2full
        vpool = ctx.enter_context(tc.tile_pool(name="vpool", bufs=2))
        oraw = ctx.enter_context(tc.tile_pool(name="oraw", bufs=2))
        rb = ctx.enter_context(tc.tile_pool(name="rb", bufs=1))
        exps_pool = ctx.enter_context(tc.tile_pool(name="exps", bufs=3))
        stat = ctx.enter_context(tc.tile_pool(name="stat", bufs=4))
        dpool = ctx.enter_context(tc.tile_pool(name="dpool", bufs=1))
        # PSUM budget (8 banks): 2 rotating [128,1024] fp32 score slots
        # (4 banks) + 2 A@V accumulator banks + 2 one-bank "aux" slots for
        # everything else (projections / transposes / FFN), so aux work
        # never queues behind the wave slots' ring.
        ps = ctx.enter_context(tc.tile_pool(name="ps", bufs=1, space="PSUM"))
        accp = ctx.enter_context(tc.tile_pool(name="accp", bufs=1, space="PSUM"))
        dram = ctx.enter_context(tc.tile_pool(name="dram", bufs=2, space="DRAM"))

        _psc = [0]

        def pstile(shape, dtype):
            """1-bank (2KB) aux PSUM slot, two rotating tags. All non-wave
            PSUM work (projections, transposes, FFN) lives here so its slot
            ring never waits on the attention waves' score slots."""
            _psc[0] += 1
            padded = [128, 512] if dtype == F32 else [128, 1024]
            return ps.tile(shape, dtype, name="pst",
                           tag=f"aux{_psc[0] % 2}",
                           padded_shape=padded)

        # ---- persistent singles ----
        identity = singles.tile([128, 128], BF16)
        make_identity(nc, identity)
        epsc = singles.tile([128, 1], F32)
        nc.vector.memset(epsc, EPS)
        h_t = singles.tile([128, NT, D], F32)
        ln0s_t = singles.tile([128, D], F32)
        ln0b_t = singles.tile([128, D], F32)
        nc.sync.dma_start(out=ln0s_t, in_=dram_bcast(ln0s_in.ap()))
        nc.sync.dma_start(out=ln0b_t, in_=dram_bcast(ln0b_in.ap()))

        def ln_stats_apply(src, dst, tiles, dst_off=0):
            """LayerNorm (stats + normalize, no scale/bias) of `src` token
            tiles [128, len(tiles), D] into dst[:, dst_off+i, :]. Stats on
            DVE; rstd Ln/Exp on ScalarE at high priority (tiny, must not
            queue behind pending exps)."""
            n = len(tiles)
            mvs = stat.tile([128, n, 2], F32, tag="mvs")
            rstd = stat.tile([128, n], F32, tag="rstd")
            for i in range(n):
                st = stat.tile([128, 6], F32, tag="bnstats")
                nc.vector.bn_stats(out=st, in_=src[:, tiles[i], :])
                nc.vector.bn_aggr(out=mvs[:, i, :], in_=st)
            with tc.high_priority():
                nc.scalar.activation(out=rstd, in_=mvs[:, :, 1],
                                     func=mybir.ActivationFunctionType.Ln,
                                     bias=epsc[:, 0:1])
                nc.scalar.activation(out=rstd, in_=rstd,
                                     func=mybir.ActivationFunctionType.Exp,
                                     scale=-0.5)
            for i, t in enumerate(tiles):
                nc.vector.tensor_scalar(
                    out=dst[:, dst_off + i, :], in0=src[:, t, :],
                    scalar1=mvs[:, i, 0:1], scalar2=rstd[:, i:i + 1],
                    op0=mybir.AluOpType.subtract, op1=mybir.AluOpType.mult)

        # ---- LN0: h = ln0(x), per w half ----
        x0 = big.tile([128, NT, D], F32, tag="x0")
        x0r = x_in.ap().rearrange("(t p) d -> p t d", p=128)
        for hf in range(2):
            nc.sync.dma_start(out=x0[:, 4 * hf:4 * hf + 4, :],
                              in_=x0r[:, 4 * hf:4 * hf + 4, :])
        for w in range(2):
            ln_stats_apply(x0, h_t, [4 * w + i for i in range(4)],
                           dst_off=4 * w)
            if not ln0_identity:
                for t in range(4 * w, 4 * w + 4):
                    nc.vector.tensor_mul(out=h_t[:, t, :], in0=h_t[:, t, :],
                                         in1=ln0s_t)
                    nc.vector.tensor_add(out=h_t[:, t, :], in0=h_t[:, t, :],
                                         in1=ln0b_t)

        # ---- layers ----
        for l in range(L):
            wt = wpool.tile([128, WFREE], BF16)
            nc.sync.dma_start(out=wt, in_=wcat[l, :, :])
            bt = bpool.tile([128, BFREE], F32)
            nc.sync.dma_start(out=bt, in_=bcat[l, :, :])
            bvbc_t = bpool.tile([128, 264], BF16, tag="bvbc")
            nc.sync.dma_start(out=bvbc_t, in_=dram_bcast(bvcat[l, :]))

            def wq_sl(ci, co):
                return wt[:, QOFF + ci * 256 + co * 128: QOFF + ci * 256 + co * 128 + 128]

            def wk_sl(ci, co):
                return wt[:, KOFF + ci * 256 + co * 128: KOFF + ci * 256 + co * 128 + 128]

            def wv_sl(ci):
                return wt[:, VOFF + ci * 264: VOFF + ci * 264 + 264]

            def wo_sl(ci, co):
                return wt[:, OOFF + ci * 256 + co * 128: OOFF + ci * 256 + co * 128 + 128]

            def w1_sl(ci, co):
                return wt[:, W1OFF + ci * 512 + co * 128: W1OFF + ci * 512 + co * 128 + 128]

            def w2_sl(ci, co):
                return wt[:, W2OFF + ci * 256 + co * 128: W2OFF + ci * 256 + co * 128 + 128]

            # per-layer persistent tiles
            x2wT = [None, None]
            qT = qp.tile([128, 2, T], BF16, tag="qt")
            kT = kv.tile([128, 2, 4, 512], BF16, tag="kt")
            x2full = kv.tile([128, 2, 4, 512], BF16, tag="x2full")
            v_t = vpool.tile([128, NKB, 264], BF16, tag="v")
            o_t = oraw.tile([128, 2, T], BF16, tag="oraw")
            denoms = dpool.tile([128, 2, 1024], F32, tag="denoms")
            rbt = rb.tile([128, 2, T], F32, tag="rb")
            rdram = dram.tile([4, 2, 1024], F32, tag="rdram")

            skip_ln1 = (l == 0) and ln0_identity

            def prep_w(w):
                """LN1(w) -> x2 -> transpose -> bounce -> AllGather(w) ->
                x2full(w-groups) -> Q(w), K(w-groups), V(w-blocks)."""
                x2w = big.tile([128, 4, D], BF16, tag="x2w")
                if skip_ln1:
                    # LN1 of an LN0-normalized vector is identity to O(eps);
                    # just downcast h for the feature-major matmuls.
                    for i in range(4):
                        nc.vector.tensor_copy(out=x2w[:, i, :],
                                              in_=h_t[:, 4 * w + i, :])
                else:
                    ln_stats_apply(h_t, x2w, [4 * w + i for i in range(4)])
                x2src, toff = x2w, 0
                # transpose own w-half to feature-major
                x2wT[w] = fm.tile([128, 2, 512], BF16, name="x2wT", tag="x2wT")
                bounce_in = dram.tile([D, 512], BF16, tag="bin")
                bounce_out = dram.tile([2 * D, 512], BF16, tag="bout")
                for c in range(2):
                    pT = pstile([128, 512], BF16)
                    for t4 in range(4):
                        nc.tensor.transpose(
                            pT[:, 128 * t4:128 * (t4 + 1)],
                            x2src[:, toff + t4, 128 * c:128 * (c + 1)],
                            identity)
                    nc.vector.tensor_copy(out=x2wT[w][:, c, :], in_=pT)
                    nc.sync.dma_start(out=bounce_in[128 * c:128 * (c + 1), :],
                                      in_=x2wT[w][:, c, :])
                nc.gpsimd.collective_compute(
                    "AllGather", mybir.AluOpType.bypass,
                    replica_groups=[[0, 1], [2, 3], [4, 5], [6, 7]],
                    ins=[bounce_in.opt()], outs=[bounce_out.opt()])
                for g in range(2):
                    for c in range(2):
                        nc.sync.dma_start(
                            out=x2full[:, c, 2 * g + w, :],
                            in_=bounce_out[D * g + 128 * c:
                                           D * g + 128 * (c + 1), :])
                # Q projection (own tokens, this w half)
                for co in range(2):
                    pq = pstile([128, 512], F32)
                    for ci in range(2):
                        nc.tensor.matmul(pq, wq_sl(ci, co), x2wT[w][:, ci, :],
                                         start=(ci == 0), stop=(ci == 1))
                    nc.vector.tensor_scalar_add(
                        out=qT[:, co, 512 * w:512 * (w + 1)], in0=pq,
                        scalar1=bt[:, BQOFF + co:BQOFF + co + 1])
                # K projection for this w's token groups (global order:
                # groups w and w+2 of kT's dim-2)
                for co in range(2):
                    for g in range(2):
                        pk = pstile([128, 512], F32)
                        for ci in range(2):
                            nc.tensor.matmul(
                                pk, wk_sl(ci, co),
                                x2full[:, ci, 2 * g + w, :],
                                start=(ci == 0), stop=(ci == 1))
                        nc.vector.tensor_scalar_add(
                            out=kT[:, co, 2 * g + w, :], in0=pk,
                            scalar1=bt[:, BKOFF + co:BKOFF + co + 1])
                # V projection (token-major) for this w's key blocks
                for kb in KB_ORDER[8 * w:8 * w + 8]:
                    pv = pstile([128, 264], F32)
                    for ci in range(2):
                        nc.tensor.matmul(
                            pv, x2full[:, ci, kb // 4, 128 * (kb % 4):
                                       128 * (kb % 4) + 128],
                            wv_sl(ci), start=(ci == 0), stop=(ci == 1))
                    nc.vector.tensor_add(out=v_t[:, kb, :], in0=pv,
                                         in1=bvbc_t)

            def attn_pass(chunk, w, mid_cb=None):
                q0 = 512 * w
                pacc0 = accp.tile([128, 512], F32, tag="acc0")
                pacc1 = accp.tile([128, 512], F32, tag="acc1")
                for ki, kb in enumerate(KB_ORDER):
                    if ki == 8 and mid_cb is not None:
                        # K/V for the second token-half must be emitted
                        # before the waves that consume them (program order
                        # carries the data deps); placing them here also
                        # ring-orders their PSUM slots between the wave
                        # slots so nothing deadlocks.
                        mid_cb()
                    sts = []
                    ets = []
                    for j in range(2):
                        sts.append(ps.tile(
                            [128, 1024], F32, name="sps", tag=f"sc{j}"))
                        ets.append(exps_pool.tile(
                            [128, 1024], BF16, name="et", tag="exps", bufs=3))
                    for hh in range(4):
                        nc.tensor.matmul(
                            sts[hh // 2][:, 512 * (hh % 2):
                                         512 * (hh % 2) + 512],
                            kT[32 * hh:32 * hh + 32, chunk, kb // 4,
                               128 * (kb % 4):128 * (kb % 4) + 128],
                            qT[32 * hh:32 * hh + 32, chunk, q0:q0 + 512],
                            start=True, stop=True,
                            tile_position=(32 * hh, 0))
                    for j in range(2):
                        nc.scalar.activation(
                            out=ets[j], in_=sts[j],
                            func=mybir.ActivationFunctionType.Exp)
                    for hh in range(4):
                        pacc = pacc0 if hh < 2 else pacc1
                        base = 64 * (hh % 2)
                        hd = 4 * chunk + hh
                        nc.tensor.matmul(
                            pacc[base:base + 33, :],
                            v_t[:, kb, 33 * hd:33 * hd + 33],
                            ets[hh // 2][:, 512 * (hh % 2):
                                         512 * (hh % 2) + 512],
                            start=(ki == 0), stop=(ki == NKB - 1),
                            tile_position=(0, base), skip_group_check=True)
                # evacuate heads: O rows + denominator rows (all DVE)
                for hh in range(4):
                    pacc = pacc0 if hh < 2 else pacc1
                    base = 64 * (hh % 2)
                    nc.vector.tensor_copy(
                        out=o_t[32 * hh:32 * hh + 32, chunk, q0:q0 + 512],
                        in_=pacc[base:base + 32, :])
                    nc.vector.tensor_copy(
                        out=denoms[32 * hh:32 * hh + 1, chunk, q0:q0 + 512],
                        in_=pacc[base + 32:base + 33, :])
                nc.sync.dma_start(out=rdram[:, chunk, q0:q0 + 512],
                                  in_=denoms[::32, chunk, q0:q0 + 512])
                for hh in range(4):
                    nc.sync.dma_start(
                        out=rbt[32 * hh:32 * hh + 32, chunk, q0:q0 + 512],
                        in_=dram_bcast(rdram[hh, chunk, q0:q0 + 512], 32))
                rsl = rbt[:, chunk, q0:q0 + 512]
                nc.vector.reciprocal(out=rsl, in_=rsl)

            def postproc_w(w):
                """normalize happened already; O-proj + residual + LN2 +
                FFN + residual for the 4 token tiles of half w."""
                q0 = 512 * w
                # ---- output projection ----
                attnU = big.tile([128, 2, 512], BF16, tag="attnU")
                for co in range(2):
                    po = pstile([128, 512], F32)
                    for ci in range(2):
                        nc.tensor.matmul(po, wo_sl(ci, co),
                                         o_t[:, ci, q0:q0 + 512],
                                         start=(ci == 0), stop=(ci == 1))
                    nc.vector.tensor_scalar_add(
                        out=attnU[:, co, :], in0=po,
                        scalar1=bt[:, BOOFF + co:BOOFF + co + 1])
                # transpose to token-major + residual add
                pT = pstile([128, 1024], BF16)
                for t4 in range(4):
                    for c in range(2):
                        nc.tensor.transpose(
                            pT[:, 256 * t4 + 128 * c:256 * t4 + 128 * (c + 1)],
                            attnU[:, c, 128 * t4:128 * (t4 + 1)], identity)
                for t4 in range(4):
                    t = 4 * w + t4
                    nc.vector.tensor_add(out=h_t[:, t, :], in0=h_t[:, t, :],
                                         in1=pT[:, 256 * t4:256 * (t4 + 1)])
                # ---- FFN (ln2 folded into w1/b1) ----
                x2f = big.tile([128, 4, D], BF16, tag="x2f")
                ln_stats_apply(h_t, x2f, [4 * w + i for i in range(4)])
                x2fT = fm.tile([128, 2, 512], BF16, tag="x2fT")
                for c in range(2):
                    pTT = pstile([128, 512], BF16)
                    for t4 in range(4):
                        nc.tensor.transpose(
                            pTT[:, 128 * t4:128 * (t4 + 1)],
                            x2f[:, t4, 128 * c:128 * (c + 1)], identity)
                    nc.vector.tensor_copy(out=x2fT[:, c, :], in_=pTT)
                h1 = big.tile([128, 4, 512], BF16, tag="h1")
                for co in range(4):
                    p1 = pstile([128, 512], F32)
                    for ci in range(2):
                        nc.tensor.matmul(p1, w1_sl(ci, co), x2fT[:, ci, :],
                                         start=(ci == 0), stop=(ci == 1))
                    # bias + relu fused on DVE
                    nc.vector.tensor_scalar(
                        out=h1[:, co, :], in0=p1,
                        scalar1=bt[:, B1OFF + co:B1OFF + co + 1], scalar2=0.0,
                        op0=mybir.AluOpType.add, op1=mybir.AluOpType.max)
                ffnU = big.tile([128, 2, 512], BF16, tag="ffnU")
                for co in range(2):
                    p2 = pstile([128, 512], F32)
                    for ci in range(4):
                        nc.tensor.matmul(p2, w2_sl(ci, co), h1[:, ci, :],
                                         start=(ci == 0), stop=(ci == 3))
                    nc.vector.tensor_scalar_add(
                        out=ffnU[:, co, :], in0=p2,
                        scalar1=bt[:, B2OFF + co:B2OFF + co + 1])
                pT2 = pstile([128, 1024], BF16)
                for t4 in range(4):
                    for c in range(2):
                        nc.tensor.transpose(
                            pT2[:, 256 * t4 + 128 * c:256 * t4 + 128 * (c + 1)],
                            ffnU[:, c, 128 * t4:128 * (t4 + 1)], identity)
                for t4 in range(4):
                    t = 4 * w + t4
                    nc.vector.tensor_add(out=h_t[:, t, :], in0=h_t[:, t, :],
                                         in1=pT2[:, 256 * t4:256 * (t4 + 1)])
                if l == L - 1:
                    yr = y_out.ap().rearrange("(t p) d -> p t d", p=128)
                    nc.sync.dma_start(out=yr[:, 4 * w:4 * w + 4, :],
                                      in_=h_t[:, 4 * w:4 * w + 4, :])

            # ---- layer emission order (priority = emission order; the
            # dataflow scheduler overlaps across it by deps) ----
            prep_w(0)
            attn_pass(0, 0, mid_cb=lambda: prep_w(1))
            attn_pass(1, 0)
            # w0 normalize after both w0 chunks
            nc.vector.tensor_mul(out=o_t[:, :, 0:512], in0=o_t[:, :, 0:512],
                                 in1=rbt[:, :, 0:512])
            attn_pass(0, 1)
            attn_pass(1, 1)
            nc.vector.tensor_mul(out=o_t[:, :, 512:1024],
                                 in0=o_t[:, :, 512:1024],
                                 in1=rbt[:, :, 512:1024])
            # postproc(w0) runs in the shadow of the w1 passes; postproc(w1)
            # is the layer tail and overlaps the next layer's prep.
            postproc_w(0)
            postproc_w(1)


# ---------------------------------------------------------------------------
# host side
# ---------------------------------------------------------------------------
_NC_CACHE = {}


def _get_nc(ln0_identity=False):
    if ln0_identity not in _NC_CACHE:
        _NC_CACHE[ln0_identity] = build_nc(ln0_identity)
    return _NC_CACHE[ln0_identity]


def _ln0_identity(inputs):
    return bool(
        np.all(np.asarray(inputs["ln0_s"], np.float32) == 1.0)
        and np.all(np.asarray(inputs["ln0_b"], np.float32) == 0.0))


def _prep_host(inputs):
    """Fold LN scales/biases + softmax scale into weights; build concat layouts."""
    f = lambda k: np.asarray(inputs[k], np.float32)
    wq, wk, wv, wo = f("wq"), f("wk"), f("wv"), f("wo")
    w1, w2 = f("w1"), f("w2")
    bq, bk, bv, bo = f("bq"), f("bk"), f("bv"), f("bo")
    b1, b2 = f("b1"), f("b2")
    l1s, l1b = f("ln1_s"), f("ln1_b")
    l2s, l2b = f("ln2_s"), f("ln2_b")

    sc = 1.0 / np.sqrt(np.float32(DK))
    wcat = np.zeros((L, 128, WFREE), np.float32)
    bcat = np.zeros((L, 128, BFREE), np.float32)
    bvcat = np.zeros((L, 264), np.float32)
    for l in range(L):
        wq_f = (l1s[l][:, None] * wq[l]) * sc
        bq_f = (l1b[l] @ wq[l] + bq[l]) * sc
        wk_f = l1s[l][:, None] * wk[l]
        bk_f = l1b[l] @ wk[l] + bk[l]
        wv_f = l1s[l][:, None] * wv[l]
        bv_f = l1b[l] @ wv[l] + bv[l]
        w1_f = l2s[l][:, None] * w1[l]
        b1_f = l2b[l] @ w1[l] + b1[l]

        # interleave wv columns into 33-wide head groups with a ones-slot
        wv_aug = np.zeros((D, 264), np.float32)
        bv_aug = np.zeros((264,), np.float32)
        for hd in range(H):
            wv_aug[:, 33 * hd:33 * hd + 32] = wv_f[:, 32 * hd:32 * hd + 32]
            bv_aug[33 * hd:33 * hd + 32] = bv_f[32 * hd:32 * hd + 32]
            bv_aug[33 * hd + 32] = 1.0  # ones column -> denominator row

        def chunks(w, width):
            n_ci = w.shape[0] // 128
            return np.concatenate(
                [w[128 * ci:128 * (ci + 1), :] for ci in range(n_ci)], axis=1)

        wcat[l, :, QOFF:QOFF + 512] = chunks(wq_f, 256)
        wcat[l, :, KOFF:KOFF + 512] = chunks(wk_f, 256)
        wcat[l, :, VOFF:VOFF + 528] = chunks(wv_aug, 264)
        wcat[l, :, OOFF:OOFF + 512] = chunks(wo[l], 256)
        wcat[l, :, W1OFF:W1OFF + 1024] = chunks(w1_f, 512)
        wcat[l, :, W2OFF:W2OFF + 1024] = chunks(w2[l], 256)

        for co in range(2):
            bcat[l, :, BQOFF + co] = bq_f[128 * co:128 * (co + 1)]
            bcat[l, :, BKOFF + co] = bk_f[128 * co:128 * (co + 1)]
            bcat[l, :, BOOFF + co] = bo[l][128 * co:128 * (co + 1)]
            bcat[l, :, B2OFF + co] = b2[l][128 * co:128 * (co + 1)]
        for co in range(4):
            bcat[l, :, B1OFF + co] = b1_f[128 * co:128 * (co + 1)]
        bvcat[l] = bv_aug

    import ml_dtypes

    return wcat.astype(ml_dtypes.bfloat16), bcat, bvcat.astype(ml_dtypes.bfloat16)


def kernel(**inputs):
    nc = _get_nc(_ln0_identity(inputs))
    wcat, bcat, bvcat = _prep_host(inputs)
    x = np.asarray(inputs["x"], np.float32)
    ln0_s = np.asarray(inputs["ln0_s"], np.float32)
    ln0_b = np.asarray(inputs["ln0_b"], np.float32)

    in_maps = []
    for c in range(NC):
        b, half = c // 2, c % 2
        in_maps.append({
            "x_sh": np.ascontiguousarray(x[b, half * T:(half + 1) * T, :]),
            "wcat": wcat, "bcat": bcat, "bvcat": bvcat,
            "ln0_s": ln0_s, "ln0_b": ln0_b,
        })

    res = run_bass_kernel_spmd(nc, in_maps, core_ids=list(range(NC)))
    out = np.zeros((B, S, D), np.float32)
    for c in range(NC):
        b, half = c // 2, c % 2
        out[b, half * T:(half + 1) * T, :] = res.results[c]["y"]
    return out


# revision 22
# speedup vs baseline: 1.2177x; 1.0523x over previous
"""Trainium2 Bass kernel for a 4-layer pre-norm transformer encoder.

Problem: B=4, S=2048, D=256, H=8 heads (DK=32), FF=512, L=4 layers, fp32.

Sharding: token-parallel over B*S across 8 cores. Core c owns batch c//2,
sequence half c%2 (1024 query tokens). Attention needs all 2048 keys of the
batch; each layer exchanges the post-LN1 activations (feature-major bf16)
within same-batch core pairs [[0,1],[2,3],[4,5],[6,7]] via TWO AllGathers,
one per query-half (w) of the tokens, so the second transfer and the K/V
projections it feeds overlap the first attention waves.

The kernel is built around the ScalarE exp wall: softmax exp over the
[keys, queries] score matrix is H*S*T = 16.8M elements/layer/core and
ScalarE (the only exp engine: 1 elem/lane/cycle @1.2GHz) is the pipeline
floor at ~147us/layer. Everything else is arranged to hide under it:
 - passes run w-major ((c0,w0),(c1,w0),(c0,w1),(c1,w1)) so the w0 tokens
   finish attention halfway through the layer; their normalize + O-proj +
   residual + LN2 + FFN ("postproc") is emitted AFTER the w1 passes so the
   dataflow Tile scheduler runs it in the PE/DVE gaps of the w1 exp stream.
 - the next layer's LN1/transpose/bounce/AllGather/Q/K/V for its w0 tokens
   depends only on postproc(w0), so it also fills the w1 shadows, and the
   next layer's first waves start right after this layer's last exp.
 - key blocks are processed in w0-blocks-first order (global token groups
   [0:512] and [1024:1536]) so waves 0-7 of every pass depend only on the
   first AllGather.
 - ALL matmul evacuations, biases, relu and softmax-denominator reciprocal
   (reciprocal_approx_fast) run on DVE; ScalarE keeps only exp and the tiny
   rstd Ln/Exp chains (emitted with high priority so they preempt the
   pending-exp queue instead of draining behind it).
 - matmul operands bf16 (full fp32 PSUM accumulation), LN scale/bias and
   the 1/sqrt(DK) folded into weights host-side; rstd via exp(-0.5*ln(v+e))
   keeps ScalarE on the natural_log_exp table set.
 - scores computed transposed S^T [keys, queries]; per wave the 4 heads'
   score matmuls stream concurrently through distinct PE row-groups
   (tile_position); A@V uses lhsT = [V | ones] so row 32 of each head
   accumulator carries the softmax denominator for free, head pairs run as
   column-tiled concurrent matmuls.
 - layer 0 skips LN1 entirely when ln0 is identity (LN of an
   already-normalized vector: rstd differs from 1 by O(eps)).
"""
import sys

sys.path.insert(0, "/opt/trn_rl_repo")

import numpy as np

import concourse.bass as bass
import concourse.mybir as mybir
import concourse.tile as tile
from concourse.bass_utils import run_bass_kernel_spmd
from concourse.masks import make_identity


# ---- problem constants (hardcoded per contract) ----
B, S, D, H, L, FF = 4, 2048, 256, 8, 4, 512
DK = D // H          # 32
EPS = 1e-5
NC = 8               # cores
T = (B * S) // NC    # 1024 own tokens per core
NT = T // 128        # 8 token tiles
SK = S               # 2048 keys
NKB = SK // 128      # 16 key blocks
F32 = mybir.dt.float32
BF16 = mybir.dt.bfloat16

# key blocks in w0-tokens-first global order: [0:512] and [1024:1536] first
KB_ORDER = [0, 1, 2, 3, 8, 9, 10, 11, 4, 5, 6, 7, 12, 13, 14, 15]

# weight-concat layout offsets (floats per partition, per layer)
QOFF, KOFF, VOFF, OOFF, W1OFF, W2OFF = 0, 512, 1024, 1552, 2064, 3088
WFREE = 4112
# bias-concat layout: bq(2) bk(2) bo(2) b1(4) b2(2) bv_bc(264)
BQOFF, BKOFF, BOOFF, B1OFF, B2OFF, BVOFF = 0, 2, 4, 6, 10, 12
BFREE = 276


def dram_bcast(ap, p=128):
    """broadcast a DRAM AP across p partitions (stride-0 leading dim)"""
    return bass.AP(tensor=ap.tensor, offset=ap.offset, ap=[[0, p]] + list(ap.ap))


def build_nc(ln0_identity=False, split_tail=True):
    nc = bass.Bass("TRN2", num_devices=NC)

    x_in = nc.declare_dram_parameter("x_sh", [T, D], F32, isOutput=False)
    wcat = nc.declare_dram_parameter("wcat", [L, 128, WFREE], BF16, isOutput=False)
    bcat = nc.declare_dram_parameter("bcat", [L, 128, BFREE], F32, isOutput=False)
    bvcat = nc.declare_dram_parameter("bvcat", [L, 264], BF16, isOutput=False)
    ln0s_in = nc.declare_dram_parameter("ln0_s", [D], F32, isOutput=False)
    ln0b_in = nc.declare_dram_parameter("ln0_b", [D], F32, isOutput=False)
    y_out = nc.declare_dram_parameter("y", [T, D], F32, isOutput=True)

    with tile.TileContext(nc) as tc:
        build_body(nc, tc, x_in, wcat, bcat, bvcat, ln0s_in, ln0b_in, y_out,
                   ln0_identity)

    if split_tail:
        _split_tail_waits(nc)
    return nc


def _split_tail_waits(nc):
    """walrus's TPB_CTRL lowering supports only one sync-wait command per
    instruction, but the TileContext kernel-tail drain aggregates one wait
    per outstanding proc lane. A chain of same-engine single-wait NoOps
    gates identically, so rewrite the tail block that way."""
    cnt = [0]

    def mk_carrier(engine, wait):
        ins = mybir.InstNoOp(name=f"waitfix-{cnt[0]}", ins=[], outs=[])
        cnt[0] += 1
        ins.engine = engine
        ins.sync_info = mybir.SyncInfo(on_wait=[wait], on_update=[])
        return ins

    def needs_split(ins):
        si = ins.sync_info
        return si is not None and len(si.on_wait) > 1

    for bb in nc.main_func.blocks:
        insts = list(bb.instructions)
        if not any(needs_split(ins) for ins in insts):
            continue
        out = []
        for ins in insts:
            si = ins.sync_info
            if needs_split(ins):
                waits = list(si.on_wait)
                for w in waits[:-1]:
                    out.append(mk_carrier(ins.engine, w))
                ins.sync_info = mybir.SyncInfo(
                    on_wait=waits[-1:], on_update=list(si.on_update)
                )
            out.append(ins)
        bb.instructions = out


def build_body(nc, tc, x_in, wcat, bcat, bvcat, ln0s_in, ln0b_in, y_out,
               ln0_identity=False):
    import contextlib

    ctx = contextlib.ExitStack()
    with ctx:
        # ---- pools ----
        singles = ctx.enter_context(tc.tile_pool(name="singles", bufs=1))
        wpool = ctx.enter_context(tc.tile_pool(name="wpool", bufs=2))
        bpool = ctx.enter_context(tc.tile_pool(name="bpool", bufs=2))
        big = ctx.enter_context(tc.tile_pool(name="big", bufs=3))
        fm = ctx.enter_context(tc.tile_pool(name="fm", bufs=2))     # x2wT
        qp = ctx.enter_context(tc.tile_pool(name="qp", bufs=2))     # qT
        kv = ctx.enter_context(tc.tile_pool(name="kv", bufs=2))     # kT/x2full
        vpool = ctx.enter_context(tc.tile_pool(name="vpool", bufs=2))
        oraw = ctx.enter_context(tc.tile_pool(name="oraw", bufs=2))
        rb = ctx.enter_context(tc.tile_pool(name="rb", bufs=1))
        exps_pool = ctx.enter_context(tc.tile_pool(name="exps", bufs=3))
        stat = ctx.enter_context(tc.tile_pool(name="stat", bufs=4))
        dpool = ctx.enter_context(tc.tile_pool(name="dpool", bufs=1))
        # PSUM budget (8 banks): 2 rotating [128,1024] fp32 score slots
        # (4 banks) + 2 A@V accumulator banks + 2 one-bank "aux" slots for
        # everything else (projections / transposes / FFN), so aux work
        # never queues behind the wave slots' ring.
        ps = ctx.enter_context(tc.tile_pool(name="ps", bufs=1, space="PSUM"))
        accp = ctx.enter_context(tc.tile_pool(name="accp", bufs=1, space="PSUM"))
        dram = ctx.enter_context(tc.tile_pool(name="dram", bufs=2, space="DRAM"))

        _psc = [0]

        def pstile(shape, dtype):
            """1-bank (2KB) aux PSUM slot, two rotating tags. All non-wave
            PSUM work (projections, transposes, FFN) lives here so its slot
            ring never waits on the attention waves' score slots."""
            _psc[0] += 1
            padded = [128, 512] if dtype == F32 else [128, 1024]
            return ps.tile(shape, dtype, name="pst",
                           tag=f"aux{_psc[0] % 2}",
                           padded_shape=padded)

        # ---- persistent singles ----
        identity = singles.tile([128, 128], BF16)
        make_identity(nc, identity)
        epsc = singles.tile([128, 1], F32)
        nc.vector.memset(epsc, EPS)
        h_t = singles.tile([128, NT, D], F32)
        ln0s_t = singles.tile([128, D], F32)
        ln0b_t = singles.tile([128, D], F32)
        nc.sync.dma_start(out=ln0s_t, in_=dram_bcast(ln0s_in.ap()))
        nc.sync.dma_start(out=ln0b_t, in_=dram_bcast(ln0b_in.ap()))

        def ln_stats_apply(src, dst, tiles, dst_off=0):
            """LayerNorm (stats + normalize, no scale/bias) of `src` token
            tiles [128, len(tiles), D] into dst[:, dst_off+i, :]. Stats on
            DVE; rstd Ln/Exp on ScalarE at high priority (tiny, must not
            queue behind pending exps)."""
            n = len(tiles)
            mvs = stat.tile([128, n, 2], F32, tag="mvs")
            rstd = stat.tile([128, n], F32, tag="rstd")
            for i in range(n):
                st = stat.tile([128, 6], F32, tag="bnstats")
                nc.vector.bn_stats(out=st, in_=src[:, tiles[i], :])
                nc.vector.bn_aggr(out=mvs[:, i, :], in_=st)
            with tc.high_priority():
                nc.scalar.activation(out=rstd, in_=mvs[:, :, 1],
                                     func=mybir.ActivationFunctionType.Ln,
                                     bias=epsc[:, 0:1])
                nc.scalar.activation(out=rstd, in_=rstd,
                                     func=mybir.ActivationFunctionType.Exp,
                                     scale=-0.5)
            for i, t in enumerate(tiles):
                nc.vector.tensor_scalar(
                    out=dst[:, dst_off + i, :], in0=src[:, t, :],
                    scalar1=mvs[:, i, 0:1], scalar2=rstd[:, i:i + 1],
                    op0=mybir.AluOpType.subtract, op1=mybir.AluOpType.mult)

        # ---- LN0: h = ln0(x), per w half ----
        x0 = big.tile([128, NT, D], F32, tag="x0")
        x0r = x_in.ap().rearrange("(t p) d -> p t d", p=128)
        for hf in range(2):
            nc.sync.dma_start(out=x0[:, 4 * hf:4 * hf + 4, :],
                              in_=x0r[:, 4 * hf:4 * hf + 4, :])
        for w in range(2):
            ln_stats_apply(x0, h_t, [4 * w + i for i in range(4)],
                           dst_off=4 * w)
            if not ln0_identity:
                for t in range(4 * w, 4 * w + 4):
                    nc.vector.tensor_mul(out=h_t[:, t, :], in0=h_t[:, t, :],
                                         in1=ln0s_t)
                    nc.vector.tensor_add(out=h_t[:, t, :], in0=h_t[:, t, :],
                                         in1=ln0b_t)

        # ---- layers ----
        class Layer:
            """Emission helper for one layer; tiles are created at prep(0)
            time so pool ring order matches emission order."""

            def __init__(self, l):
                self.l = l
                self.started = False

            def start(self):
                self.started = True
                self.wt = wpool.tile([128, WFREE], BF16, name="wt")
                nc.sync.dma_start(out=self.wt, in_=wcat[self.l, :, :])
                self.bt = bpool.tile([128, BFREE], F32, name="bt")
                nc.sync.dma_start(out=self.bt, in_=bcat[self.l, :, :])
                self.bvbc = bpool.tile([128, 264], BF16, name="bvbc",
                                       tag="bvbc")
                nc.sync.dma_start(out=self.bvbc,
                                  in_=dram_bcast(bvcat[self.l, :]))
                self.qT = qp.tile([128, 2, T], BF16, name="qT", tag="qt")
                self.kT = kv.tile([128, 2, 4, 512], BF16, name="kT", tag="kt")
                self.x2full = kv.tile([128, 2, 4, 512], BF16, name="x2full",
                                      tag="x2full")
                self.v_t = vpool.tile([128, NKB, 264], BF16, name="v_t",
                                      tag="v")
                self.o_t = oraw.tile([128, 2, T], BF16, name="o_t",
                                     tag="oraw")
                self.denoms = dpool.tile([128, 2, 1024], F32, name="denoms",
                                         tag="denoms")
                self.rbt = rb.tile([128, 2, T], F32, name="rbt", tag="rb")
                self.rdram = dram.tile([4, 2, 1024], F32, name="rdram",
                                       tag="rdram")
                self.x2wT = [None, None]

            def wsl(self, off, ci, co, wd, nco=2):
                base = off + ci * nco * wd + co * wd
                return self.wt[:, base:base + wd]

            def prep(self, w):
                """LN1(w) -> x2 -> transpose -> bounce -> AllGather(w) ->
                x2full(w-groups) -> Q(w), K(w-groups), V(w-blocks)."""
                if not self.started:
                    self.start()
                l, bt, kT, x2full = self.l, self.bt, self.kT, self.x2full
                x2w = big.tile([128, 4, D], BF16, name="x2w", tag="x2w")
                if (l == 0) and ln0_identity:
                    # LN1 of an LN0-normalized vector is identity to O(eps);
                    # just downcast h for the feature-major matmuls.
                    for i in range(4):
                        nc.vector.tensor_copy(out=x2w[:, i, :],
                                              in_=h_t[:, 4 * w + i, :])
                else:
                    ln_stats_apply(h_t, x2w, [4 * w + i for i in range(4)])
                # transpose own w-half to feature-major
                x2wT = fm.tile([128, 2, 512], BF16, name="x2wT", tag="x2wT")
                self.x2wT[w] = x2wT
                bounce_in = dram.tile([D, 512], BF16, name="bounce_in",
                                      tag="bin")
                bounce_out = dram.tile([2 * D, 512], BF16, name="bounce_out",
                                       tag="bout")
                for c in range(2):
                    pT = pstile([128, 512], BF16)
                    for t4 in range(4):
                        nc.tensor.transpose(
                            pT[:, 128 * t4:128 * (t4 + 1)],
                            x2w[:, t4, 128 * c:128 * (c + 1)], identity)
                    nc.vector.tensor_copy(out=x2wT[:, c, :], in_=pT)
                    nc.sync.dma_start(out=bounce_in[128 * c:128 * (c + 1), :],
                                      in_=x2wT[:, c, :])
                nc.gpsimd.collective_compute(
                    "AllGather", mybir.AluOpType.bypass,
                    replica_groups=[[0, 1], [2, 3], [4, 5], [6, 7]],
                    ins=[bounce_in.opt()], outs=[bounce_out.opt()])
                for g in range(2):
                    for c in range(2):
                        nc.sync.dma_start(
                            out=x2full[:, c, 2 * g + w, :],
                            in_=bounce_out[D * g + 128 * c:
                                           D * g + 128 * (c + 1), :])
                # Q projection (own tokens, this w half)
                for co in range(2):
                    pq = pstile([128, 512], F32)
                    for ci in range(2):
                        nc.tensor.matmul(pq, self.wsl(QOFF, ci, co, 128),
                                         x2wT[:, ci, :],
                                         start=(ci == 0), stop=(ci == 1))
                    nc.vector.tensor_scalar_add(
                        out=self.qT[:, co, 512 * w:512 * (w + 1)], in0=pq,
                        scalar1=bt[:, BQOFF + co:BQOFF + co + 1])
                # K projection for this w's token groups (global order:
                # groups w and w+2 of kT's dim-2)
                for co in range(2):
                    for g in range(2):
                        pk = pstile([128, 512], F32)
                        for ci in range(2):
                            nc.tensor.matmul(
                                pk, self.wsl(KOFF, ci, co, 128),
                                x2full[:, ci, 2 * g + w, :],
                                start=(ci == 0), stop=(ci == 1))
                        nc.vector.tensor_scalar_add(
                            out=kT[:, co, 2 * g + w, :], in0=pk,
                            scalar1=bt[:, BKOFF + co:BKOFF + co + 1])
                # V projection (token-major) for this w's key blocks
                for kb in KB_ORDER[8 * w:8 * w + 8]:
                    pv = pstile([128, 264], F32)
                    for ci in range(2):
                        nc.tensor.matmul(
                            pv, x2full[:, ci, kb // 4, 128 * (kb % 4):
                                       128 * (kb % 4) + 128],
                            self.wt[:, VOFF + ci * 264:VOFF + ci * 264 + 264],
                            start=(ci == 0), stop=(ci == 1))
                    nc.vector.tensor_add(out=self.v_t[:, kb, :], in0=pv,
                                         in1=self.bvbc)

            def attn_pass(self, chunk, w, mid_cb=None):
                q0 = 512 * w
                qT, kT, v_t, o_t = self.qT, self.kT, self.v_t, self.o_t
                denoms, rbt, rdram = self.denoms, self.rbt, self.rdram
                pacc0 = accp.tile([128, 512], F32, name="pacc0", tag="acc0")
                pacc1 = accp.tile([128, 512], F32, name="pacc1", tag="acc1")
                for ki, kb in enumerate(KB_ORDER):
                    if ki == 8 and mid_cb is not None:
                        # K/V for the second token-half must be emitted
                        # before the waves that consume them (program order
                        # carries the data deps). Deprioritized so it only
                        # gap-fills the exp stream.
                        with tc.high_priority(offset=-700):
                            mid_cb()
                    sts = []
                    ets = []
                    for j in range(2):
                        sts.append(ps.tile(
                            [128, 1024], F32, name="sps", tag=f"sc{j}"))
                        ets.append(exps_pool.tile(
                            [128, 1024], BF16, name="et", tag="exps", bufs=3))
                    for hh in range(4):
                        nc.tensor.matmul(
                            sts[hh // 2][:, 512 * (hh % 2):
                                         512 * (hh % 2) + 512],
                            kT[32 * hh:32 * hh + 32, chunk, kb // 4,
                               128 * (kb % 4):128 * (kb % 4) + 128],
                            qT[32 * hh:32 * hh + 32, chunk, q0:q0 + 512],
                            start=True, stop=True,
                            tile_position=(32 * hh, 0))
                    for j in range(2):
                        nc.scalar.activation(
                            out=ets[j], in_=sts[j],
                            func=mybir.ActivationFunctionType.Exp)
                    for hh in range(4):
                        pacc = pacc0 if hh < 2 else pacc1
                        base = 64 * (hh % 2)
                        hd = 4 * chunk + hh
                        nc.tensor.matmul(
                            pacc[base:base + 33, :],
                            v_t[:, kb, 33 * hd:33 * hd + 33],
                            ets[hh // 2][:, 512 * (hh % 2):
                                         512 * (hh % 2) + 512],
                            start=(ki == 0), stop=(ki == NKB - 1),
                            tile_position=(0, base), skip_group_check=True)
                # evacuate heads: O rows + denominator rows (all DVE)
                for hh in range(4):
                    pacc = pacc0 if hh < 2 else pacc1
                    base = 64 * (hh % 2)
                    nc.vector.tensor_copy(
                        out=o_t[32 * hh:32 * hh + 32, chunk, q0:q0 + 512],
                        in_=pacc[base:base + 32, :])
                    nc.vector.tensor_copy(
                        out=denoms[32 * hh:32 * hh + 1, chunk, q0:q0 + 512],
                        in_=pacc[base + 32:base + 33, :])
                nc.sync.dma_start(out=rdram[:, chunk, q0:q0 + 512],
                                  in_=denoms[::32, chunk, q0:q0 + 512])
                for hh in range(4):
                    nc.sync.dma_start(
                        out=rbt[32 * hh:32 * hh + 32, chunk, q0:q0 + 512],
                        in_=dram_bcast(rdram[hh, chunk, q0:q0 + 512], 32))
                rsl = rbt[:, chunk, q0:q0 + 512]
                nc.vector.reciprocal(out=rsl, in_=rsl)

            def normalize(self, w):
                q0 = 512 * w
                nc.vector.tensor_mul(out=self.o_t[:, :, q0:q0 + 512],
                                     in0=self.o_t[:, :, q0:q0 + 512],
                                     in1=self.rbt[:, :, q0:q0 + 512])

            def postproc(self, w):
                """O-proj + residual + LN2 + FFN + residual for the 4 token
                tiles of half w (o_t already normalized)."""
                l, bt, o_t = self.l, self.bt, self.o_t
                q0 = 512 * w
                # ---- output projection ----
                attnU = big.tile([128, 2, 512], BF16, name="attnU",
                                 tag="attnU")
                for co in range(2):
                    po = pstile([128, 512], F32)
                    for ci in range(2):
                        nc.tensor.matmul(po, self.wsl(OOFF, ci, co, 128),
                                         o_t[:, ci, q0:q0 + 512],
                                         start=(ci == 0), stop=(ci == 1))
                    nc.vector.tensor_scalar_add(
                        out=attnU[:, co, :], in0=po,
                        scalar1=bt[:, BOOFF + co:BOOFF + co + 1])
                # transpose to token-major + residual add
                pT = pstile([128, 1024], BF16)
                for t4 in range(4):
                    for c in range(2):
                        nc.tensor.transpose(
                            pT[:, 256 * t4 + 128 * c:256 * t4 + 128 * (c + 1)],
                            attnU[:, c, 128 * t4:128 * (t4 + 1)], identity)
                for t4 in range(4):
                    t = 4 * w + t4
                    nc.vector.tensor_add(out=h_t[:, t, :], in0=h_t[:, t, :],
                                         in1=pT[:, 256 * t4:256 * (t4 + 1)])
                # ---- FFN (ln2 folded into w1/b1) ----
                x2f = big.tile([128, 4, D], BF16, name="x2f", tag="x2f")
                ln_stats_apply(h_t, x2f, [4 * w + i for i in range(4)])
                x2fT = fm.tile([128, 2, 512], BF16, name="x2fT", tag="x2fT")
                for c in range(2):
                    pTT = pstile([128, 512], BF16)
                    for t4 in range(4):
                        nc.tensor.transpose(
                            pTT[:, 128 * t4:128 * (t4 + 1)],
                            x2f[:, t4, 128 * c:128 * (c + 1)], identity)
                    nc.vector.tensor_copy(out=x2fT[:, c, :], in_=pTT)
                h1 = big.tile([128, 4, 512], BF16, name="h1", tag="h1")
                for co in range(4):
                    p1 = pstile([128, 512], F32)
                    for ci in range(2):
                        nc.tensor.matmul(p1, self.wsl(W1OFF, ci, co, 128, 4),
                                         x2fT[:, ci, :],
                                         start=(ci == 0), stop=(ci == 1))
                    # bias + relu fused on DVE
                    nc.vector.tensor_scalar(
                        out=h1[:, co, :], in0=p1,
                        scalar1=bt[:, B1OFF + co:B1OFF + co + 1], scalar2=0.0,
                        op0=mybir.AluOpType.add, op1=mybir.AluOpType.max)
                ffnU = big.tile([128, 2, 512], BF16, name="ffnU", tag="ffnU")
                for co in range(2):
                    p2 = pstile([128, 512], F32)
                    for ci in range(4):
                        nc.tensor.matmul(p2, self.wsl(W2OFF, ci, co, 128),
                                         h1[:, ci, :],
                                         start=(ci == 0), stop=(ci == 3))
                    nc.vector.tensor_scalar_add(
                        out=ffnU[:, co, :], in0=p2,
                        scalar1=bt[:, B2OFF + co:B2OFF + co + 1])
                pT2 = pstile([128, 1024], BF16)
                for t4 in range(4):
                    for c in range(2):
                        nc.tensor.transpose(
                            pT2[:, 256 * t4 + 128 * c:256 * t4 + 128 * (c + 1)],
                            ffnU[:, c, 128 * t4:128 * (t4 + 1)], identity)
                for t4 in range(4):
                    t = 4 * w + t4
                    nc.vector.tensor_add(out=h_t[:, t, :], in0=h_t[:, t, :],
                                         in1=pT2[:, 256 * t4:256 * (t4 + 1)])
                if l == L - 1:
                    yr = y_out.ap().rearrange("(t p) d -> p t d", p=128)
                    nc.sync.dma_start(out=yr[:, 4 * w:4 * w + 4, :],
                                      in_=h_t[:, 4 * w:4 * w + 4, :])

        # ---- emission order. Priority = emission order; the dataflow
        # scheduler overlaps across it by deps, but data deps and the
        # per-tag PSUM slot rings follow emission order, so:
        #  - prep(1) is emitted mid-first-pass (before the waves that read
        #    its K/V), deprioritized to gap-fill;
        #  - postproc(0) after the w1 passes so it runs in their shadow;
        #  - the NEXT layer's prep(0) before this layer's postproc(1), so
        #    the next layer's first waves start right after our last exp
        #    while postproc(1) drains in their shadow. ----
        layers = [Layer(l) for l in range(L)]
        layers[0].prep(0)
        for l in range(L):
            cur = layers[l]
            cur.attn_pass(0, 0, mid_cb=lambda cur=cur: cur.prep(1))
            cur.attn_pass(1, 0)
            cur.normalize(0)
            cur.attn_pass(0, 1)
            cur.attn_pass(1, 1)
            cur.normalize(1)
            cur.postproc(0)
            if l + 1 < L:
                layers[l + 1].prep(0)
            cur.postproc(1)


# ---------------------------------------------------------------------------
# host side
# ---------------------------------------------------------------------------
_NC_CACHE = {}


def _get_nc(ln0_identity=False):
    if ln0_identity not in _NC_CACHE:
        _NC_CACHE[ln0_identity] = build_nc(ln0_identity)
    return _NC_CACHE[ln0_identity]


def _ln0_identity(inputs):
    return bool(
        np.all(np.asarray(inputs["ln0_s"], np.float32) == 1.0)
        and np.all(np.asarray(inputs["ln0_b"], np.float32) == 0.0))


def _prep_host(inputs):
    """Fold LN scales/biases + softmax scale into weights; build concat layouts."""
    f = lambda k: np.asarray(inputs[k], np.float32)
    wq, wk, wv, wo = f("wq"), f("wk"), f("wv"), f("wo")
    w1, w2 = f("w1"), f("w2")
    bq, bk, bv, bo = f("bq"), f("bk"), f("bv"), f("bo")
    b1, b2 = f("b1"), f("b2")
    l1s, l1b = f("ln1_s"), f("ln1_b")
    l2s, l2b = f("ln2_s"), f("ln2_b")

    sc = 1.0 / np.sqrt(np.float32(DK))
    wcat = np.zeros((L, 128, WFREE), np.float32)
    bcat = np.zeros((L, 128, BFREE), np.float32)
    bvcat = np.zeros((L, 264), np.float32)
    for l in range(L):
        wq_f = (l1s[l][:, None] * wq[l]) * sc
        bq_f = (l1b[l] @ wq[l] + bq[l]) * sc
        wk_f = l1s[l][:, None] * wk[l]
        bk_f = l1b[l] @ wk[l] + bk[l]
        wv_f = l1s[l][:, None] * wv[l]
        bv_f = l1b[l] @ wv[l] + bv[l]
        w1_f = l2s[l][:, None] * w1[l]
        b1_f = l2b[l] @ w1[l] + b1[l]

        # interleave wv columns into 33-wide head groups with a ones-slot
        wv_aug = np.zeros((D, 264), np.float32)
        bv_aug = np.zeros((264,), np.float32)
        for hd in range(H):
            wv_aug[:, 33 * hd:33 * hd + 32] = wv_f[:, 32 * hd:32 * hd + 32]
            bv_aug[33 * hd:33 * hd + 32] = bv_f[32 * hd:32 * hd + 32]
            bv_aug[33 * hd + 32] = 1.0  # ones column -> denominator row

        def chunks(w, width):
            n_ci = w.shape[0] // 128
            return np.concatenate(
                [w[128 * ci:128 * (ci + 1), :] for ci in range(n_ci)], axis=1)

        wcat[l, :, QOFF:QOFF + 512] = chunks(wq_f, 256)
        wcat[l, :, KOFF:KOFF + 512] = chunks(wk_f, 256)
        wcat[l, :, VOFF:VOFF + 528] = chunks(wv_aug, 264)
        wcat[l, :, OOFF:OOFF + 512] = chunks(wo[l], 256)
        wcat[l, :, W1OFF:W1OFF + 1024] = chunks(w1_f, 512)
        wcat[l, :, W2OFF:W2OFF + 1024] = chunks(w2[l], 256)

        for co in range(2):
            bcat[l, :, BQOFF + co] = bq_f[128 * co:128 * (co + 1)]
            bcat[l, :, BKOFF + co] = bk_f[128 * co:128 * (co + 1)]
            bcat[l, :, BOOFF + co] = bo[l][128 * co:128 * (co + 1)]
            bcat[l, :, B2OFF + co] = b2[l][128 * co:128 * (co + 1)]
        for co in range(4):
            bcat[l, :, B1OFF + co] = b1_f[128 * co:128 * (co + 1)]
        bvcat[l] = bv_aug

    import ml_dtypes

    return wcat.astype(ml_dtypes.bfloat16), bcat, bvcat.astype(ml_dtypes.bfloat16)


def kernel(**inputs):
    nc = _get_nc(_ln0_identity(inputs))
    wcat, bcat, bvcat = _prep_host(inputs)
    x = np.asarray(inputs["x"], np.float32)
    ln0_s = np.asarray(inputs["ln0_s"], np.float32)
    ln0_b = np.asarray(inputs["ln0_b"], np.float32)

    in_maps = []
    for c in range(NC):
        b, half = c // 2, c % 2
        in_maps.append({
            "x_sh": np.ascontiguousarray(x[b, half * T:(half + 1) * T, :]),
            "wcat": wcat, "bcat": bcat, "bvcat": bvcat,
            "ln0_s": ln0_s, "ln0_b": ln0_b,
        })

    res = run_bass_kernel_spmd(nc, in_maps, core_ids=list(range(NC)))
    out = np.zeros((B, S, D), np.float32)
    for c in range(NC):
        b, half = c // 2, c % 2
        out[b, half * T:(half + 1) * T, :] = res.results[c]["y"]
    return out


# revision 34
# speedup vs baseline: 1.2417x; 1.0197x over previous
"""Trainium2 Bass kernel for a 4-layer pre-norm transformer encoder.

Problem: B=4, S=2048, D=256, H=8 heads (DK=32), FF=512, L=4 layers, fp32.

Sharding: token-parallel over B*S across 8 cores. Core c owns batch c//2,
sequence half c%2 (1024 query tokens). Attention needs all 2048 keys of the
batch; each layer exchanges the post-LN1 activations (feature-major bf16)
within same-batch core pairs [[0,1],[2,3],[4,5],[6,7]] via TWO AllGathers,
one per query-half (w) of the tokens, so the second transfer and the K/V
projections it feeds overlap the first attention waves.

The kernel is built around the ScalarE exp wall: softmax exp over the
[keys, queries] score matrix is H*S*T = 16.8M elements/layer/core and
ScalarE (the only exp engine: 1 elem/lane/cycle @1.2GHz) is the pipeline
floor at ~147us/layer. Everything else is arranged to hide under it:
 - passes run w-major ((c0,w0),(c1,w0),(c0,w1),(c1,w1)) so the w0 tokens
   finish attention halfway through the layer; their normalize + O-proj +
   residual + LN2 + FFN ("postproc") is emitted AFTER the w1 passes so the
   dataflow Tile scheduler runs it in the PE/DVE gaps of the w1 exp stream.
 - the next layer's LN1/transpose/bounce/AllGather/Q/K/V for its w0 tokens
   depends only on postproc(w0), so it also fills the w1 shadows, and the
   next layer's first waves start right after this layer's last exp.
 - key blocks are processed in w0-blocks-first order (global token groups
   [0:512] and [1024:1536]) so waves 0-7 of every pass depend only on the
   first AllGather.
 - ALL matmul evacuations, biases, relu and softmax-denominator reciprocal
   (reciprocal_approx_fast) run on DVE; ScalarE keeps only exp and the tiny
   rstd Ln/Exp chains (emitted with high priority so they preempt the
   pending-exp queue instead of draining behind it).
 - matmul operands bf16 (full fp32 PSUM accumulation), LN scale/bias and
   the 1/sqrt(DK) folded into weights host-side; rstd via exp(-0.5*ln(v+e))
   keeps ScalarE on the natural_log_exp table set.
 - scores computed transposed S^T [keys, queries]; per wave the 4 heads'
   score matmuls stream concurrently through distinct PE row-groups
   (tile_position); A@V uses lhsT = [V | ones] so row 32 of each head
   accumulator carries the softmax denominator for free, head pairs run as
   column-tiled concurrent matmuls.
 - layer 0 skips LN1 entirely when ln0 is identity (LN of an
   already-normalized vector: rstd differs from 1 by O(eps)).
"""
import sys

sys.path.insert(0, "/opt/trn_rl_repo")

import numpy as np

import concourse.bass as bass
import concourse.mybir as mybir
import concourse.tile as tile
from concourse.bass_utils import run_bass_kernel_spmd
from concourse.masks import make_identity


# ---- problem constants (hardcoded per contract) ----
B, S, D, H, L, FF = 4, 2048, 256, 8, 4, 512
DK = D // H          # 32
EPS = 1e-5
NC = 8               # cores
T = (B * S) // NC    # 1024 own tokens per core
NT = T // 128        # 8 token tiles
SK = S               # 2048 keys
NKB = SK // 128      # 16 key blocks
F32 = mybir.dt.float32
BF16 = mybir.dt.bfloat16

# key blocks in w0-tokens-first global order: [0:512] and [1024:1536] first
KB_ORDER = [0, 1, 2, 3, 8, 9, 10, 11, 4, 5, 6, 7, 12, 13, 14, 15]

# weight-concat layout offsets (floats per partition, per layer)
QOFF, KOFF, VOFF, OOFF, W1OFF, W2OFF = 0, 512, 1024, 1552, 2064, 3088
WFREE = 4112
# bias-concat layout: bq(2) bk(2) bo(2) b1(4) b2(2) bv_bc(264)
BQOFF, BKOFF, BOOFF, B1OFF, B2OFF, BVOFF = 0, 2, 4, 6, 10, 12
BFREE = 276


def dram_bcast(ap, p=128):
    """broadcast a DRAM AP across p partitions (stride-0 leading dim)"""
    return bass.AP(tensor=ap.tensor, offset=ap.offset, ap=[[0, p]] + list(ap.ap))


def build_nc(ln0_identity=False, split_tail=True):
    nc = bass.Bass("TRN2", num_devices=NC)

    x_in = nc.declare_dram_parameter("x_sh", [T, D], F32, isOutput=False)
    wcat = nc.declare_dram_parameter("wcat", [L, 128, WFREE], BF16, isOutput=False)
    bcat = nc.declare_dram_parameter("bcat", [L, 128, BFREE], F32, isOutput=False)
    bvcat = nc.declare_dram_parameter("bvcat", [L, 264], BF16, isOutput=False)
    ln0s_in = nc.declare_dram_parameter("ln0_s", [D], F32, isOutput=False)
    ln0b_in = nc.declare_dram_parameter("ln0_b", [D], F32, isOutput=False)
    y_out = nc.declare_dram_parameter("y", [T, D], F32, isOutput=True)

    with tile.TileContext(nc) as tc:
        build_body(nc, tc, x_in, wcat, bcat, bvcat, ln0s_in, ln0b_in, y_out,
                   ln0_identity)

    if split_tail:
        _split_tail_waits(nc)
    return nc


def _split_tail_waits(nc):
    """walrus's TPB_CTRL lowering supports only one sync-wait command per
    instruction, but the TileContext kernel-tail drain aggregates one wait
    per outstanding proc lane. A chain of same-engine single-wait NoOps
    gates identically, so rewrite the tail block that way."""
    cnt = [0]

    def mk_carrier(engine, wait):
        ins = mybir.InstNoOp(name=f"waitfix-{cnt[0]}", ins=[], outs=[])
        cnt[0] += 1
        ins.engine = engine
        ins.sync_info = mybir.SyncInfo(on_wait=[wait], on_update=[])
        return ins

    def needs_split(ins):
        si = ins.sync_info
        return si is not None and len(si.on_wait) > 1

    for bb in nc.main_func.blocks:
        insts = list(bb.instructions)
        if not any(needs_split(ins) for ins in insts):
            continue
        out = []
        for ins in insts:
            si = ins.sync_info
            if needs_split(ins):
                waits = list(si.on_wait)
                for w in waits[:-1]:
                    out.append(mk_carrier(ins.engine, w))
                ins.sync_info = mybir.SyncInfo(
                    on_wait=waits[-1:], on_update=list(si.on_update)
                )
            out.append(ins)
        bb.instructions = out


def build_body(nc, tc, x_in, wcat, bcat, bvcat, ln0s_in, ln0b_in, y_out,
               ln0_identity=False):
    import contextlib

    ctx = contextlib.ExitStack()
    with ctx:
        # ---- pools ----
        singles = ctx.enter_context(tc.tile_pool(name="singles", bufs=1))
        wpool = ctx.enter_context(tc.tile_pool(name="wpool", bufs=2))
        bpool = ctx.enter_context(tc.tile_pool(name="bpool", bufs=2))
        big = ctx.enter_context(tc.tile_pool(name="big", bufs=3))
        fm = ctx.enter_context(tc.tile_pool(name="fm", bufs=2))     # x2wT
        qp = ctx.enter_context(tc.tile_pool(name="qp", bufs=2))     # qT
        kv = ctx.enter_context(tc.tile_pool(name="kv", bufs=2))     # kT/x2full
        vpool = ctx.enter_context(tc.tile_pool(name="vpool", bufs=2))
        oraw = ctx.enter_context(tc.tile_pool(name="oraw", bufs=2))
        rb = ctx.enter_context(tc.tile_pool(name="rb", bufs=1))
        exps_pool = ctx.enter_context(tc.tile_pool(name="exps", bufs=3))
        stat = ctx.enter_context(tc.tile_pool(name="stat", bufs=4))
        dpool = ctx.enter_context(tc.tile_pool(name="dpool", bufs=1))
        # PSUM budget (8 banks): 2 rotating [128,1024] fp32 wave slots
        # (2 banks each; one slot = 4 heads x 256 queries) + 1 A@V
        # accumulator bank shared by all 4 heads (has_written clears by
        # whole bank: the first matmul of a pass uses start=True, the other
        # heads' first matmuls overwrite-on-pending-zero) + 3 one-bank
        # "aux" slots for everything else (projections / transposes /
        # FFN), so aux work never queues behind the wave slots' ring.
        ps = ctx.enter_context(tc.tile_pool(name="ps", bufs=1, space="PSUM"))
        accp = ctx.enter_context(tc.tile_pool(name="accp", bufs=1, space="PSUM"))
        dram = ctx.enter_context(tc.tile_pool(name="dram", bufs=2, space="DRAM"))

        _psc = [0]

        def pstile(shape, dtype):
            """1-bank (2KB) aux PSUM slot, two rotating tags. All non-wave
            PSUM work (projections, transposes, FFN) lives here so its slot
            ring never waits on the attention waves' score slots."""
            _psc[0] += 1
            padded = [128, 512] if dtype == F32 else [128, 1024]
            return ps.tile(shape, dtype, name="pst",
                           tag=f"aux{_psc[0] % 2}",
                           padded_shape=padded)

        # ---- persistent singles ----
        identity = singles.tile([128, 128], BF16)
        make_identity(nc, identity)
        epsc = singles.tile([128, 1], F32)
        nc.vector.memset(epsc, EPS)
        h_t = singles.tile([128, NT, D], F32)
        ln0s_t = singles.tile([128, D], F32)
        ln0b_t = singles.tile([128, D], F32)
        nc.sync.dma_start(out=ln0s_t, in_=dram_bcast(ln0s_in.ap()))
        nc.sync.dma_start(out=ln0b_t, in_=dram_bcast(ln0b_in.ap()))

        def ln_stats_apply(src, dst, tiles, dst_off=0):
            """LayerNorm (stats + normalize, no scale/bias) of `src` token
            tiles [128, len(tiles), D] into dst[:, dst_off+i, :]. Stats on
            DVE; rstd Ln/Exp on ScalarE at high priority (tiny, must not
            queue behind pending exps)."""
            n = len(tiles)
            mvs = stat.tile([128, n, 2], F32, tag="mvs")
            rstd = stat.tile([128, n], F32, tag="rstd")
            for i in range(n):
                st = stat.tile([128, 6], F32, tag="bnstats")
                nc.vector.bn_stats(out=st, in_=src[:, tiles[i], :])
                nc.vector.bn_aggr(out=mvs[:, i, :], in_=st)
            with tc.high_priority():
                nc.scalar.activation(out=rstd, in_=mvs[:, :, 1],
                                     func=mybir.ActivationFunctionType.Ln,
                                     bias=epsc[:, 0:1])
                nc.scalar.activation(out=rstd, in_=rstd,
                                     func=mybir.ActivationFunctionType.Exp,
                                     scale=-0.5)
            for i, t in enumerate(tiles):
                nc.vector.tensor_scalar(
                    out=dst[:, dst_off + i, :], in0=src[:, t, :],
                    scalar1=mvs[:, i, 0:1], scalar2=rstd[:, i:i + 1],
                    op0=mybir.AluOpType.subtract, op1=mybir.AluOpType.mult)

        # ---- LN0: h = ln0(x), per w half ----
        x0 = big.tile([128, NT, D], F32, tag="x0")
        x0r = x_in.ap().rearrange("(t p) d -> p t d", p=128)
        for hf in range(2):
            nc.sync.dma_start(out=x0[:, 4 * hf:4 * hf + 4, :],
                              in_=x0r[:, 4 * hf:4 * hf + 4, :])
        for w in range(2):
            ln_stats_apply(x0, h_t, [4 * w + i for i in range(4)],
                           dst_off=4 * w)
            if not ln0_identity:
                for t in range(4 * w, 4 * w + 4):
                    nc.vector.tensor_mul(out=h_t[:, t, :], in0=h_t[:, t, :],
                                         in1=ln0s_t)
                    nc.vector.tensor_add(out=h_t[:, t, :], in0=h_t[:, t, :],
                                         in1=ln0b_t)

        # ---- layers ----
        class Layer:
            """Emission helper for one layer; tiles are created at prep(0)
            time so pool ring order matches emission order."""

            def __init__(self, l):
                self.l = l
                self.started = False

            def start(self):
                self.started = True
                self.wt = wpool.tile([128, WFREE], BF16, name="wt")
                nc.sync.dma_start(out=self.wt, in_=wcat[self.l, :, :])
                self.bt = bpool.tile([128, BFREE], F32, name="bt")
                nc.sync.dma_start(out=self.bt, in_=bcat[self.l, :, :])
                self.bvbc = bpool.tile([128, 264], BF16, name="bvbc",
                                       tag="bvbc")
                nc.sync.dma_start(out=self.bvbc,
                                  in_=dram_bcast(bvcat[self.l, :]))
                self.qT = qp.tile([128, 2, T], BF16, name="qT", tag="qt")
                self.kT = kv.tile([128, 2, 4, 512], BF16, name="kT", tag="kt")
                self.x2full = kv.tile([128, 2, 4, 512], BF16, name="x2full",
                                      tag="x2full")
                self.v_t = vpool.tile([128, NKB, 264], BF16, name="v_t",
                                      tag="v")
                self.o_t = oraw.tile([128, 2, T], BF16, name="o_t",
                                     tag="oraw")
                self.denoms = dpool.tile([128, 2, 1024], F32, name="denoms",
                                         tag="denoms")
                self.rbt = rb.tile([128, 2, T], F32, name="rbt", tag="rb")
                self.rdram = dram.tile([4, 2, 1024], F32, name="rdram",
                                       tag="rdram")
                self.x2wT = [None, None]

            def wsl(self, off, ci, co, wd, nco=2):
                base = off + ci * nco * wd + co * wd
                return self.wt[:, base:base + wd]

            def prep(self, w):
                """LN1(w) -> x2 -> transpose -> bounce -> AllGather(w) ->
                x2full(w-groups) -> Q(w), K(w-groups), V(w-blocks)."""
                if not self.started:
                    self.start()
                l, bt, kT, x2full = self.l, self.bt, self.kT, self.x2full
                x2w = big.tile([128, 4, D], BF16, name="x2w", tag="x2w")
                if (l == 0) and ln0_identity:
                    # LN1 of an LN0-normalized vector is identity to O(eps);
                    # just downcast h for the feature-major matmuls.
                    for i in range(4):
                        nc.vector.tensor_copy(out=x2w[:, i, :],
                                              in_=h_t[:, 4 * w + i, :])
                else:
                    ln_stats_apply(h_t, x2w, [4 * w + i for i in range(4)])
                # transpose own w-half to feature-major
                x2wT = fm.tile([128, 2, 512], BF16, name="x2wT", tag="x2wT")
                self.x2wT[w] = x2wT
                bounce_in = dram.tile([D, 512], BF16, name="bounce_in",
                                      tag="bin")
                bounce_out = dram.tile([2 * D, 512], BF16, name="bounce_out",
                                       tag="bout")
                for c in range(2):
                    pT = pstile([128, 512], BF16)
                    for t4 in range(4):
                        nc.tensor.transpose(
                            pT[:, 128 * t4:128 * (t4 + 1)],
                            x2w[:, t4, 128 * c:128 * (c + 1)], identity)
                    nc.vector.tensor_copy(out=x2wT[:, c, :], in_=pT)
                    nc.sync.dma_start(out=bounce_in[128 * c:128 * (c + 1), :],
                                      in_=x2wT[:, c, :])
                nc.gpsimd.collective_compute(
                    "AllGather", mybir.AluOpType.bypass,
                    replica_groups=[[0, 1], [2, 3], [4, 5], [6, 7]],
                    ins=[bounce_in.opt()], outs=[bounce_out.opt()])
                for g in range(2):
                    for c in range(2):
                        nc.sync.dma_start(
                            out=x2full[:, c, 2 * g + w, :],
                            in_=bounce_out[D * g + 128 * c:
                                           D * g + 128 * (c + 1), :])
                # Q projection (own tokens, this w half)
                for co in range(2):
                    pq = pstile([128, 512], F32)
                    for ci in range(2):
                        nc.tensor.matmul(pq, self.wsl(QOFF, ci, co, 128),
                                         x2wT[:, ci, :],
                                         start=(ci == 0), stop=(ci == 1))
                    nc.vector.tensor_scalar_add(
                        out=self.qT[:, co, 512 * w:512 * (w + 1)], in0=pq,
                        scalar1=bt[:, BQOFF + co:BQOFF + co + 1])
                # K projection for this w's token groups (global order:
                # groups w and w+2 of kT's dim-2)
                for co in range(2):
                    for g in range(2):
                        pk = pstile([128, 512], F32)
                        for ci in range(2):
                            nc.tensor.matmul(
                                pk, self.wsl(KOFF, ci, co, 128),
                                x2full[:, ci, 2 * g + w, :],
                                start=(ci == 0), stop=(ci == 1))
                        nc.vector.tensor_scalar_add(
                            out=kT[:, co, 2 * g + w, :], in0=pk,
                            scalar1=bt[:, BKOFF + co:BKOFF + co + 1])
                # V projection (token-major) for this w's key blocks
                for kb in KB_ORDER[8 * w:8 * w + 8]:
                    pv = pstile([128, 264], F32)
                    for ci in range(2):
                        nc.tensor.matmul(
                            pv, x2full[:, ci, kb // 4, 128 * (kb % 4):
                                       128 * (kb % 4) + 128],
                            self.wt[:, VOFF + ci * 264:VOFF + ci * 264 + 264],
                            start=(ci == 0), stop=(ci == 1))
                    nc.vector.tensor_add(out=self.v_t[:, kb, :], in0=pv,
                                         in1=self.bvbc)

            def attn_pass(self, chunk, w, mid_cb=None):
                """One pass = 4 heads (chunk) x 512 queries (half w).
                Per wave: 4 score matmuls into two [128,1024] score tiles
                (each 512-col slice is a full PSUM bank: concurrent PE
                streams must hit distinct banks), 2 exps, 4 A@V matmuls
                col-tiled pairwise into two accumulator banks."""
                q0 = 512 * w
                qT, kT, v_t, o_t = self.qT, self.kT, self.v_t, self.o_t
                denoms, rbt, rdram = self.denoms, self.rbt, self.rdram
                pacc0 = accp.tile([128, 512], F32, name="pacc0", tag="acc0")
                pacc1 = accp.tile([128, 512], F32, name="pacc1", tag="acc1")
                for ki, kb in enumerate(KB_ORDER):
                    if ki == 8 and mid_cb is not None:
                        # K/V for the second token-half must be emitted
                        # before the waves that consume them (program order
                        # carries the data deps). Deprioritized so it only
                        # gap-fills the exp stream.
                        with tc.high_priority(offset=-700):
                            mid_cb()
                    sts = []
                    ets = []
                    for j in range(2):
                        sts.append(ps.tile(
                            [128, 1024], F32, name="sps", tag=f"sc{j}"))
                        ets.append(exps_pool.tile(
                            [128, 1024], BF16, name="et", tag="exps", bufs=3))
                    for hh in range(4):
                        nc.tensor.matmul(
                            sts[hh // 2][:, 512 * (hh % 2):
                                         512 * (hh % 2) + 512],
                            kT[32 * hh:32 * hh + 32, chunk, kb // 4,
                               128 * (kb % 4):128 * (kb % 4) + 128],
                            qT[32 * hh:32 * hh + 32, chunk, q0:q0 + 512],
                            start=True, stop=True,
                            tile_position=(32 * hh, 0))
                    for j in range(2):
                        nc.scalar.activation(
                            out=ets[j], in_=sts[j],
                            func=mybir.ActivationFunctionType.Exp)
                    for hh in range(4):
                        pacc = pacc0 if hh < 2 else pacc1
                        base = 64 * (hh % 2)
                        hd = 4 * chunk + hh
                        nc.tensor.matmul(
                            pacc[base:base + 33, :],
                            v_t[:, kb, 33 * hd:33 * hd + 33],
                            ets[hh // 2][:, 512 * (hh % 2):
                                         512 * (hh % 2) + 512],
                            start=(ki == 0), stop=(ki == NKB - 1),
                            tile_position=(0, base), skip_group_check=True)
                # evacuate heads: O rows + denominator rows (all DVE)
                for hh in range(4):
                    pacc = pacc0 if hh < 2 else pacc1
                    base = 64 * (hh % 2)
                    nc.vector.tensor_copy(
                        out=o_t[32 * hh:32 * hh + 32, chunk, q0:q0 + 512],
                        in_=pacc[base:base + 32, :])
                    nc.vector.tensor_copy(
                        out=denoms[32 * hh:32 * hh + 1, chunk, q0:q0 + 512],
                        in_=pacc[base + 32:base + 33, :])
                nc.sync.dma_start(out=rdram[:, chunk, q0:q0 + 512],
                                  in_=denoms[::32, chunk, q0:q0 + 512])
                for hh in range(4):
                    nc.sync.dma_start(
                        out=rbt[32 * hh:32 * hh + 32, chunk, q0:q0 + 512],
                        in_=dram_bcast(rdram[hh, chunk, q0:q0 + 512], 32))
                rsl = rbt[:, chunk, q0:q0 + 512]
                nc.vector.reciprocal(out=rsl, in_=rsl)

            def normalize(self, w):
                q0 = 512 * w
                nc.vector.tensor_mul(out=self.o_t[:, :, q0:q0 + 512],
                                     in0=self.o_t[:, :, q0:q0 + 512],
                                     in1=self.rbt[:, :, q0:q0 + 512])

            def postproc(self, w):
                """O-proj + residual + LN2 + FFN + residual for the 4 token
                tiles of half w (o_t already normalized)."""
                l, bt, o_t = self.l, self.bt, self.o_t
                q0 = 512 * w
                # ---- output projection ----
                attnU = big.tile([128, 2, 512], BF16, name="attnU",
                                 tag="attnU")
                for co in range(2):
                    po = pstile([128, 512], F32)
                    for ci in range(2):
                        nc.tensor.matmul(po, self.wsl(OOFF, ci, co, 128),
                                         o_t[:, ci, q0:q0 + 512],
                                         start=(ci == 0), stop=(ci == 1))
                    nc.vector.tensor_scalar_add(
                        out=attnU[:, co, :], in0=po,
                        scalar1=bt[:, BOOFF + co:BOOFF + co + 1])
                # transpose to token-major + residual add
                pT = pstile([128, 1024], BF16)
                for t4 in range(4):
                    for c in range(2):
                        nc.tensor.transpose(
                            pT[:, 256 * t4 + 128 * c:256 * t4 + 128 * (c + 1)],
                            attnU[:, c, 128 * t4:128 * (t4 + 1)], identity)
                for t4 in range(4):
                    t = 4 * w + t4
                    nc.vector.tensor_add(out=h_t[:, t, :], in0=h_t[:, t, :],
                                         in1=pT[:, 256 * t4:256 * (t4 + 1)])
                # ---- FFN (ln2 folded into w1/b1) ----
                x2f = big.tile([128, 4, D], BF16, name="x2f", tag="x2f")
                ln_stats_apply(h_t, x2f, [4 * w + i for i in range(4)])
                x2fT = fm.tile([128, 2, 512], BF16, name="x2fT", tag="x2fT")
                for c in range(2):
                    pTT = pstile([128, 512], BF16)
                    for t4 in range(4):
                        nc.tensor.transpose(
                            pTT[:, 128 * t4:128 * (t4 + 1)],
                            x2f[:, t4, 128 * c:128 * (c + 1)], identity)
                    nc.vector.tensor_copy(out=x2fT[:, c, :], in_=pTT)
                h1 = big.tile([128, 4, 512], BF16, name="h1", tag="h1")
                for co in range(4):
                    p1 = pstile([128, 512], F32)
                    for ci in range(2):
                        nc.tensor.matmul(p1, self.wsl(W1OFF, ci, co, 128, 4),
                                         x2fT[:, ci, :],
                                         start=(ci == 0), stop=(ci == 1))
                    # bias + relu fused on DVE
                    nc.vector.tensor_scalar(
                        out=h1[:, co, :], in0=p1,
                        scalar1=bt[:, B1OFF + co:B1OFF + co + 1], scalar2=0.0,
                        op0=mybir.AluOpType.add, op1=mybir.AluOpType.max)
                ffnU = big.tile([128, 2, 512], BF16, name="ffnU", tag="ffnU")
                for co in range(2):
                    p2 = pstile([128, 512], F32)
                    for ci in range(4):
                        nc.tensor.matmul(p2, self.wsl(W2OFF, ci, co, 128),
                                         h1[:, ci, :],
                                         start=(ci == 0), stop=(ci == 3))
                    nc.vector.tensor_scalar_add(
                        out=ffnU[:, co, :], in0=p2,
                        scalar1=bt[:, B2OFF + co:B2OFF + co + 1])
                pT2 = pstile([128, 1024], BF16)
                for t4 in range(4):
                    for c in range(2):
                        nc.tensor.transpose(
                            pT2[:, 256 * t4 + 128 * c:256 * t4 + 128 * (c + 1)],
                            ffnU[:, c, 128 * t4:128 * (t4 + 1)], identity)
                for t4 in range(4):
                    t = 4 * w + t4
                    nc.vector.tensor_add(out=h_t[:, t, :], in0=h_t[:, t, :],
                                         in1=pT2[:, 256 * t4:256 * (t4 + 1)])
                if l == L - 1:
                    yr = y_out.ap().rearrange("(t p) d -> p t d", p=128)
                    nc.sync.dma_start(out=yr[:, 4 * w:4 * w + 4, :],
                                      in_=h_t[:, 4 * w:4 * w + 4, :])

        # ---- emission order. Priority = emission order; the dataflow
        # scheduler overlaps across it by deps, but data deps and the
        # per-tag PSUM slot rings follow emission order, so:
        #  - prep(1) is emitted mid-first-pass (before the waves that read
        #    its K/V), deprioritized to gap-fill;
        #  - postproc(0) after the w1 passes so it runs in their shadow;
        #  - the NEXT layer's prep(0) before this layer's postproc(1), so
        #    the next layer's first waves start right after our last exp
        #    while postproc(1) drains in their shadow. ----
        layers = [Layer(l) for l in range(L)]
        layers[0].prep(0)
        for l in range(L):
            cur = layers[l]
            cur.attn_pass(0, 0, mid_cb=lambda cur=cur: cur.prep(1))
            cur.attn_pass(1, 0)
            cur.normalize(0)
            cur.attn_pass(0, 1)
            cur.attn_pass(1, 1)
            cur.normalize(1)
            cur.postproc(0)
            if l + 1 < L:
                layers[l + 1].prep(0)
            cur.postproc(1)


# ---------------------------------------------------------------------------
# host side
# ---------------------------------------------------------------------------
_NC_CACHE = {}


def _get_nc(ln0_identity=False):
    if ln0_identity not in _NC_CACHE:
        _NC_CACHE[ln0_identity] = build_nc(ln0_identity)
    return _NC_CACHE[ln0_identity]


def _ln0_identity(inputs):
    return bool(
        np.all(np.asarray(inputs["ln0_s"], np.float32) == 1.0)
        and np.all(np.asarray(inputs["ln0_b"], np.float32) == 0.0))


def _prep_host(inputs):
    """Fold LN scales/biases + softmax scale into weights; build concat layouts."""
    f = lambda k: np.asarray(inputs[k], np.float32)
    wq, wk, wv, wo = f("wq"), f("wk"), f("wv"), f("wo")
    w1, w2 = f("w1"), f("w2")
    bq, bk, bv, bo = f("bq"), f("bk"), f("bv"), f("bo")
    b1, b2 = f("b1"), f("b2")
    l1s, l1b = f("ln1_s"), f("ln1_b")
    l2s, l2b = f("ln2_s"), f("ln2_b")

    sc = 1.0 / np.sqrt(np.float32(DK))
    wcat = np.zeros((L, 128, WFREE), np.float32)
    bcat = np.zeros((L, 128, BFREE), np.float32)
    bvcat = np.zeros((L, 264), np.float32)
    for l in range(L):
        wq_f = (l1s[l][:, None] * wq[l]) * sc
        bq_f = (l1b[l] @ wq[l] + bq[l]) * sc
        wk_f = l1s[l][:, None] * wk[l]
        bk_f = l1b[l] @ wk[l] + bk[l]
        wv_f = l1s[l][:, None] * wv[l]
        bv_f = l1b[l] @ wv[l] + bv[l]
        w1_f = l2s[l][:, None] * w1[l]
        b1_f = l2b[l] @ w1[l] + b1[l]

        # interleave wv columns into 33-wide head groups with a ones-slot
        wv_aug = np.zeros((D, 264), np.float32)
        bv_aug = np.zeros((264,), np.float32)
        for hd in range(H):
            wv_aug[:, 33 * hd:33 * hd + 32] = wv_f[:, 32 * hd:32 * hd + 32]
            bv_aug[33 * hd:33 * hd + 32] = bv_f[32 * hd:32 * hd + 32]
            bv_aug[33 * hd + 32] = 1.0  # ones column -> denominator row

        def chunks(w, width):
            n_ci = w.shape[0] // 128
            return np.concatenate(
                [w[128 * ci:128 * (ci + 1), :] for ci in range(n_ci)], axis=1)

        wcat[l, :, QOFF:QOFF + 512] = chunks(wq_f, 256)
        wcat[l, :, KOFF:KOFF + 512] = chunks(wk_f, 256)
        wcat[l, :, VOFF:VOFF + 528] = chunks(wv_aug, 264)
        wcat[l, :, OOFF:OOFF + 512] = chunks(wo[l], 256)
        wcat[l, :, W1OFF:W1OFF + 1024] = chunks(w1_f, 512)
        wcat[l, :, W2OFF:W2OFF + 1024] = chunks(w2[l], 256)

        for co in range(2):
            bcat[l, :, BQOFF + co] = bq_f[128 * co:128 * (co + 1)]
            bcat[l, :, BKOFF + co] = bk_f[128 * co:128 * (co + 1)]
            bcat[l, :, BOOFF + co] = bo[l][128 * co:128 * (co + 1)]
            bcat[l, :, B2OFF + co] = b2[l][128 * co:128 * (co + 1)]
        for co in range(4):
            bcat[l, :, B1OFF + co] = b1_f[128 * co:128 * (co + 1)]
        bvcat[l] = bv_aug

    import ml_dtypes

    return wcat.astype(ml_dtypes.bfloat16), bcat, bvcat.astype(ml_dtypes.bfloat16)


def kernel(**inputs):
    nc = _get_nc(_ln0_identity(inputs))
    wcat, bcat, bvcat = _prep_host(inputs)
    x = np.asarray(inputs["x"], np.float32)
    ln0_s = np.asarray(inputs["ln0_s"], np.float32)
    ln0_b = np.asarray(inputs["ln0_b"], np.float32)

    in_maps = []
    for c in range(NC):
        b, half = c // 2, c % 2
        in_maps.append({
            "x_sh": np.ascontiguousarray(x[b, half * T:(half + 1) * T, :]),
            "wcat": wcat, "bcat": bcat, "bvcat": bvcat,
            "ln0_s": ln0_s, "ln0_b": ln0_b,
        })

    res = run_bass_kernel_spmd(nc, in_maps, core_ids=list(range(NC)))
    out = np.zeros((B, S, D), np.float32)
    for c in range(NC):
        b, half = c // 2, c % 2
        out[b, half * T:(half + 1) * T, :] = res.results[c]["y"]
    return out
